# revision 1
# baseline (speedup 1.0000x reference)
"""Trainium2 Bass kernel for nn_AttentionLayer (B=4, N=4096, D=1024).

Reference computation:
  nx = layernorm(x)
  h  = nx @ expand                       # [B,N,4352]
  q  = h[:, :128] ; k = h[:, 128:256]
  linear = h[:, 256:2304]; pre_gelu = h[:, 2304:4352]
  gated  = linear * gelu(pre_gelu)       # exact erf gelu
  local  = gated[:, :1024]; v = gated[:, 1024:2048]
  mask[i,j] = j<=i ? sigmoid((j-i)+pbm) : -inf
  attn = softmax(q k^T / sqrt(128) + mask) @ v
  out  = x + concat([local, attn]) @ project

Sharding (8 cores, SPMD): batch b -> core pair (2b, 2b+1).  Per pair,
512-row query blocks interleave for causal load balance: even core owns
blocks {0,3,4,7}, odd owns {1,2,5,6}.  Each core computes LN + expand for
its OWN 2048 rows only; k/v of the other half arrive via four pairwise
AllGathers (one per 512-row chunk, issued as each chunk's k/v lands in
HBM so the wire time pipelines under the remaining expand).  The kv slot
order is the fixed pair order [even-core blocks | odd-core blocks], the
same on both cores, so the SPMD attention schedule is uniform:
q-slot i attends a fixed slot set (2/4/6/8 slots), and causality +
position bias are enforced by a host-precomputed multiplicative mask
expM = causal ? exp(sigmoid(j-i+pbm)) : 0; the device computes
P = exp(qk) * expM and normalizes by the row sum (no max subtraction:
logits are O(1) after layernorm + xavier weights).  LN stats/centering
are software-pipelined one chunk ahead of the expand matmuls.

All matmuls in bf16 (fp32 matmul is 4x slower on TRN2), psum accumulation f32.
"""

import math

import numpy as np
import ml_dtypes

import concourse.bass as bass
import concourse.mybir as mybir
from concourse import bacc
import concourse.tile as tile
from concourse.bass_utils import run_bass_kernel_spmd

BF16 = mybir.dt.bfloat16
F32 = mybir.dt.float32
AF = mybir.ActivationFunctionType

B, N, D = 4, 4096, 1024
QK = 128
E = 2048
NB = 1024          # query/key block
R = N              # kv rows per core
RO = 2048          # own query rows per core
DCH = D // 128     # 8 contraction chunks
NT = 512           # matmul free-dim tile
W2 = 2176          # 128 (q or k) + 1024 (linear) + 1024 (gelu) cols

LAST_RESULTS = None  # set by kernel(); test harness reads exec_time_ns


def _build_nc(trace_friendly_names=True):
    nc = bacc.Bacc(None)

    xt = nc.declare_dram_parameter("xt", [D, RO], BF16, isOutput=False)
    xo = nc.declare_dram_parameter("xo", [RO, D], F32, isOutput=False)
    wkv = nc.declare_dram_parameter("wkv", [D, W2], BF16, isOutput=False)
    wql = nc.declare_dram_parameter("wql", [D, W2], BF16, isOutput=False)
    wproj = nc.declare_dram_parameter("wproj", [E, D], BF16, isOutput=False)
    msk = nc.declare_dram_parameter("msk", [10240, NT], BF16, isOutput=False)
    out = nc.declare_dram_parameter("out", [RO, D], F32, isOutput=True)

    RCH = R // NT           # 8 row chunks of 512
    ROCH = RO // NT         # 4 own row chunks

    with tile.TileContext(nc) as tc:
        with tc.tile_pool(name="const", bufs=1) as cpool:
            ones128 = cpool.tile([128, 1], BF16)
            nc.vector.memset(ones128[:], 1.0)
            ones1 = cpool.tile([1, 128], BF16)
            nc.vector.memset(ones1[:], 1.0)

            with tc.tile_pool(name="dram", bufs=1, space="DRAM") as dpool:
                kv_own = [dpool.tile([128, 4 * 1024 + (RO if r == 3 else 0)], BF16,
                                     name=f"kv_own_{r}") for r in range(4)]
                kv_all = [dpool.tile([2 * 128, 4 * 1024 + (RO if r == 3 else 0)], BF16,
                                     name=f"kv_all_{r}") for r in range(4)]

                with tc.tile_pool(name="persist", bufs=1) as ppool:
                    kT_sb = ppool.tile([128, R], BF16)         # k^T, hT layout
                    qT_sb = ppool.tile([128, RO], BF16)        # q^T (prescaled 1/sqrt(qk))
                    localT_sb = ppool.tile([128, 8 * RO], BF16)  # [lc_ch][128, 2048]

                    # ---------------- Phase 1+2: expand ----------------
                    with tc.tile_pool(name="wkv_p", bufs=1) as wkvp, \
                         tc.tile_pool(name="wql_p", bufs=1) as wqlp, \
                         tc.tile_pool(name="ex_stream", bufs=4) as estream, \
                         tc.tile_pool(name="ex_work", bufs=3) as ework, \
                         tc.tile_pool(name="st_work", bufs=2) as swork, \
                         tc.tile_pool(name="ex_psum", bufs=5, space="PSUM") as epsum, \
                         tc.tile_pool(name="st_psum", bufs=2, space="PSUM") as spsum:
                        wkv_sb = wkvp.tile([128, DCH * W2], BF16)
                        for dch in range(DCH):
                            nc.sync.dma_start(wkv_sb[:, dch * W2:(dch + 1) * W2],
                                              wkv[dch * 128:(dch + 1) * 128, :])
                        wql_sb = wqlp.tile([128, DCH * W2], BF16)
                        for dch in range(DCH):
                            nc.sync.dma_start(wql_sb[:, dch * W2:(dch + 1) * W2],
                                              wql[dch * 128:(dch + 1) * 128, :])

                        def stats_chain(rch):
                            """DMA x^T tiles for rch and compute LN scale/shift
                            broadcast tiles.  Emitted one iteration ahead so the
                            DVE math hides under the previous chunk's expand."""
                            rs = rch * NT
                            xts = []
                            for dch in range(DCH):
                                t = estream.tile([128, NT], BF16, tag="xt_e", bufs=10,
                                                 name=f"xt_{rch}_{dch}")
                                nc.sync.dma_start(t[:], xt[dch * 128:(dch + 1) * 128, rs:rs + NT])
                                xts.append(t)
                            mu_ps = spsum.tile([1, NT], F32, tag="stat", name=f"mu_ps_{rch}")
                            sq_ps = spsum.tile([1, NT], F32, tag="stat", name=f"sq_ps_{rch}")
                            # accumulate the 8 d-chunks on DVE (bf16 2x mode), then a
                            # single partition-sum matmul per stat instead of 8 each
                            acc_mu = estream.tile([128, NT], BF16, tag="acc_mu", bufs=3,
                                                  name=f"accmu_{rch}")
                            acc_sq = estream.tile([128, NT], BF16, tag="acc_sq", bufs=3,
                                                  name=f"accsq_{rch}")
                            sq_prev = estream.tile([128, NT], BF16, tag="sq_s", bufs=3,
                                                    name=f"sq_{rch}_0")
                            nc.scalar.activation(sq_prev[:], xts[0][:], AF.Square)
                            nc.vector.tensor_add(acc_mu[:], xts[0][:], xts[1][:])
                            for dch in range(1, DCH):
                                sqt = estream.tile([128, NT], BF16, tag="sq_s", bufs=3,
                                                   name=f"sq_{rch}_{dch}")
                                nc.scalar.activation(sqt[:], xts[dch][:], AF.Square)
                                if dch == 1:
                                    nc.vector.tensor_add(acc_sq[:], sq_prev[:], sqt[:])
                                else:
                                    nc.vector.tensor_add(acc_sq[:], acc_sq[:], sqt[:])
                                if dch >= 2:
                                    nc.vector.tensor_add(acc_mu[:], acc_mu[:], xts[dch][:])
                            nc.tensor.matmul(mu_ps[:], ones128[:], acc_mu[:],
                                             start=True, stop=True)
                            nc.tensor.matmul(sq_ps[:], ones128[:], acc_sq[:],
                                             start=True, stop=True)
                            mu = swork.tile([1, NT], F32, tag="st_mu", bufs=1, name=f"mu_{rch}")
                            e2 = swork.tile([1, NT], F32, tag="st_e2", bufs=1, name=f"e2_{rch}")
                            scr = swork.tile([1, NT], F32, tag="st_scr", bufs=1, name=f"scr_{rch}")
                            nc.vector.tensor_scalar_mul(mu[:], mu_ps[:], 1.0 / D)
                            nc.vector.tensor_scalar_mul(e2[:], sq_ps[:], 1.0 / D)
                            nc.vector.tensor_mul(scr[:], mu[:], mu[:])
                            nc.vector.tensor_sub(e2[:], e2[:], scr[:])
                            nc.vector.tensor_scalar_add(e2[:], e2[:], 1e-5)
                            nc.scalar.activation(e2[:], e2[:], AF.Sqrt)
                            nc.vector.reciprocal_approx_fast(scr[:], e2[:])          # rstd
                            nc.vector.scalar_tensor_tensor(
                                mu[:], mu[:], -1.0, scr[:],
                                op0=mybir.AluOpType.mult, op1=mybir.AluOpType.mult)  # -mu*rstd
                            rstd16 = swork.tile([1, NT], BF16, tag="st_r16", name=f"r16_{rch}")
                            sneg16 = swork.tile([1, NT], BF16, tag="st_s16", name=f"s16_{rch}")
                            nc.vector.tensor_copy(rstd16[:], scr[:])
                            nc.vector.tensor_copy(sneg16[:], mu[:])
                            return xts, rstd16, sneg16

                        def bcast_chain(rch, rstd16, sneg16):
                            # rank-1 broadcast [1,NT] -> [128,NT]; emitted mid-expand
                            # of the previous chunk so the DVE math above is hidden
                            bps = spsum.tile([128, NT], F32, tag="bcast", bufs=1, name=f"bps_{rch}")
                            nc.tensor.matmul(bps[:], ones1[:], rstd16[:], start=True, stop=True)
                            rstd_bt = swork.tile([128, NT], BF16, tag="rbt", bufs=3,
                                                 name=f"rbt_{rch}")
                            nc.vector.tensor_copy(rstd_bt[:], bps[:])
                            bps2 = spsum.tile([128, NT], F32, tag="bcast", bufs=1, name=f"bps2_{rch}")
                            nc.tensor.matmul(bps2[:], ones1[:], sneg16[:], start=True, stop=True)
                            sneg_bt = swork.tile([128, NT], BF16, tag="sbt", bufs=3,
                                                 name=f"sbt_{rch}")
                            nc.vector.tensor_copy(sneg_bt[:], bps2[:])
                            return rstd_bt, sneg_bt

                        # own chunks (0-3, ~70us expand each) interleaved with
                        # foreign chunks (4-7, ~30us) so every next-chunk stats
                        # chain has a long expand to hide under
                        rch_order = [0, 1, 2, 3]

                        def center_chain(rch, xts, rstd_bt, sneg_bt):
                            # x'' = x*rstd - mu*rstd; emitted mid-way through the
                            # PREVIOUS chunk's expand so the DVE work is hidden
                            xpp = []
                            for dch in range(DCH):
                                xc = estream.tile([128, NT], BF16, tag="xpp", bufs=34,
                                                  name=f"xpp_{rch}_{dch}")
                                nc.vector.tensor_mul(xc[:], xts[dch][:], rstd_bt[:])
                                nc.vector.tensor_add(xc[:], xc[:], sneg_bt[:])
                                xpp.append(xc)
                            return xpp

                        def v_group(rch, xpp, ms):
                            for m in ms:
                                vlin = ework.tile([128, E // 2], BF16, tag="vlin")
                                vgel = ework.tile([128, E // 2], BF16, tag="vgel")
                                for vc in range(4):
                                    vps = epsum.tile([128, NT], F32, tag="mm")
                                    if vc < 2:
                                        woff = 128 + vc * NT
                                    else:
                                        woff = 1152 + (vc - 2) * NT
                                    for dch in range(DCH):
                                        nc.tensor.matmul(
                                            vps[:],
                                            xpp[dch][:, m * 128:(m + 1) * 128],
                                            wkv_sb[:, dch * W2 + woff:dch * W2 + woff + NT],
                                            start=(dch == 0), stop=(dch == DCH - 1))
                                    if vc < 2:
                                        nc.vector.tensor_copy(vlin[:, vc * NT:(vc + 1) * NT], vps[:])
                                    else:
                                        nc.scalar.activation(vgel[:, (vc - 2) * NT:(vc - 1) * NT],
                                                             vps[:], AF.Gelu)
                                vv = ework.tile([128, E // 2], BF16, tag="vv")
                                nc.vector.tensor_mul(vv[:], vlin[:], vgel[:])
                                nc.sync.dma_start(kv_own[rch][:, m * 1024:(m + 1) * 1024], vv[:])

                        st0 = stats_chain(rch_order[0])
                        bt0 = bcast_chain(rch_order[0], st0[1], st0[2])
                        xpp_stash = center_chain(rch_order[0], st0[0], bt0[0], bt0[1])
                        xpps = {}
                        NOWN = 4
                        for oi, rch in enumerate(rch_order):
                            rs = rch * NT
                            xpp = xpp_stash
                            xpps[rch] = xpp
                            if oi + 1 < NOWN:
                                nxt = stats_chain(rch_order[oi + 1])
                            # k^T own (hT layout) -> DRAM bounce for the AllGather
                            kps = epsum.tile([128, NT], F32, tag="mm")
                            for dch in range(DCH):
                                nc.tensor.matmul(kps[:], wkv_sb[:, dch * W2:dch * W2 + 128],
                                                 xpp[dch][:],
                                                 start=(dch == 0), stop=(dch == DCH - 1))
                            kout = ework.tile([128, NT], BF16, tag="kout", bufs=2,
                                              name=f"kout_{rch}")
                            nc.vector.tensor_copy(kout[:], kps[:])
                            nc.sync.dma_start(kv_own[3][:, 4096 + rs:4096 + rs + NT], kout[:])
                            v_group(rch, xpp, (0, 1))
                            if oi + 1 < NOWN:
                                nbt = bcast_chain(rch_order[oi + 1], nxt[1], nxt[2])
                                xpp_stash = center_chain(rch_order[oi + 1], nxt[0],
                                                         nbt[0], nbt[1])
                            v_group(rch, xpp, (2, 3))
                            # AllGather for this chunk's v (and, for the last, all k):
                            # issued as soon as the chunk is in HBM so the wire time
                            # pipelines under the remaining expand
                            nc.gpsimd.collective_compute(
                                "AllGather",
                                mybir.AluOpType.bypass,
                                replica_groups=[[0, 1], [2, 3], [4, 5], [6, 7]],
                                ins=[kv_own[rch].opt()],
                                outs=[kv_all[rch].opt()],
                            )
                        for r in range(2):
                            nc.sync.dma_start(
                                kT_sb[:, r * RO:(r + 1) * RO],
                                kv_all[3][r * 128:(r + 1) * 128, 4096:4096 + RO])
                        # loop2: q + local expand (covers the AllGather latency)
                        for rch in rch_order:
                            rs = rch * NT
                            xpp = xpps[rch]
                            qps = epsum.tile([128, NT], F32, tag="mm")
                            for dch in range(DCH):
                                nc.tensor.matmul(qps[:], wql_sb[:, dch * W2:dch * W2 + 128],
                                                 xpp[dch][:],
                                                 start=(dch == 0), stop=(dch == DCH - 1))
                            nc.vector.tensor_copy(qT_sb[:, rs:rs + NT], qps[:])
                            for lc in range(8):
                                lps = epsum.tile([128, NT], F32, tag="mm")
                                gps = epsum.tile([128, NT], F32, tag="mm")
                                for dch in range(DCH):
                                    nc.tensor.matmul(
                                        lps[:],
                                        wql_sb[:, dch * W2 + 128 + lc * 128:dch * W2 + 256 + lc * 128],
                                        xpp[dch][:],
                                        start=(dch == 0), stop=(dch == DCH - 1))
                                for dch in range(DCH):
                                    nc.tensor.matmul(
                                        gps[:],
                                        wql_sb[:, dch * W2 + 1152 + lc * 128:dch * W2 + 1280 + lc * 128],
                                        xpp[dch][:],
                                        start=(dch == 0), stop=(dch == DCH - 1))
                                lgel = ework.tile([128, NT], BF16, tag="lgel")
                                nc.scalar.activation(lgel[:], gps[:], AF.Gelu)
                                llin = ework.tile([128, NT], BF16, tag="llin")
                                nc.vector.tensor_copy(llin[:], lps[:])
                                nc.vector.tensor_mul(
                                    localT_sb[:, lc * RO + rs:lc * RO + rs + NT],
                                    llin[:], lgel[:])

                    # ---------------- Phase 3: attention ----------------
                    # attnT pool encloses phase 4 too (read by project)
                    with tc.tile_pool(name="attnT_p", bufs=1) as apool:
                      attnT_sb = apool.tile([128, 8 * RO], BF16)  # [vc_ch][128, 2048]
                      proj_sb = apool.tile([128, 16 * D], BF16)   # prefetched during attention
                      for _cch in range(16):
                          nc.sync.dma_start(proj_sb[:, _cch * D:(_cch + 1) * D],
                                            wproj[_cch * 128:(_cch + 1) * 128, :])
                      with tc.tile_pool(name="psb_p", bufs=1) as psbp, \
                         tc.tile_pool(name="at_stream", bufs=6) as astream, \
                         tc.tile_pool(name="at_work", bufs=2) as awork, \
                         tc.tile_pool(name="at_psum", bufs=4, space="PSUM") as apsum, \
                         tc.tile_pool(name="av_psum", bufs=2, space="PSUM") as avpsum:

                        def attention(qi, kr_slots, moff):
                            qcol = qi * NT
                            nkr = len(kr_slots) * 4  # 128-row kr chunks
                            psb = psbp.tile([128, 32 * NT], BF16, tag="psb")
                            den_ps = apsum.tile([1, NT], F32, tag="den", bufs=1)
                            den_acc = awork.tile([128, NT], BF16, tag="den_acc", bufs=2,
                                                 name=f"den_acc_{qcol}")
                            for i, krs in enumerate(kr_slots):
                                for j in range(4):
                                    ti = i * 4 + j
                                    kr0 = krs * NT + j * 128
                                    mr0 = moff + i * NT + j * 128
                                    pt_ps = apsum.tile([128, NT], F32, tag="pt", bufs=3)
                                    nc.tensor.matmul(pt_ps[:], kT_sb[:, kr0:kr0 + 128],
                                                     qT_sb[:, qcol:qcol + NT],
                                                     start=True, stop=True)
                                    pe = awork.tile([128, NT], BF16, tag="pe", bufs=4)
                                    nc.scalar.activation(pe[:], pt_ps[:], AF.Exp)
                                    mt = astream.tile([128, NT], BF16, tag="mt", bufs=10)
                                    nc.sync.dma_start(mt[:], msk[mr0:mr0 + 128, :])
                                    nc.vector.tensor_mul(psb[:, ti * NT:(ti + 1) * NT], pe[:], mt[:])
                                    if ti == 0:
                                        nc.vector.tensor_copy(den_acc[:], psb[:, 0:NT])
                                    else:
                                        nc.vector.tensor_add(den_acc[:], den_acc[:],
                                                             psb[:, ti * NT:(ti + 1) * NT])
                            nc.tensor.matmul(den_ps[:], ones128[:], den_acc[:],
                                             start=True, stop=True)
                            # AV first: the denom reciprocal chain is emitted
                            # after the first AV group so it hides under PE work.
                            rd_b = None
                            for g in range(2):
                                avs = [avpsum.tile([128, NT], F32, tag="av", bufs=4,
                                                   name=f"av{g}_{_i}")
                                       for _i in range(4)]
                                for i, krs in enumerate(kr_slots):
                                    for j in range(4):
                                        ti = i * 4 + j
                                        rb = krs * 4 + j
                                        vt = astream.tile([128, NT], BF16, tag="vt", bufs=10)
                                        gslot, vj = rb // 4, rb % 4
                                        vrank, vbuf = (0, gslot) if gslot < 4 else (1, gslot - 4)
                                        nc.sync.dma_start(
                                            vt[:], kv_all[vbuf][vrank * 128:(vrank + 1) * 128,
                                                               vj * 1024 + g * NT:vj * 1024 + (g + 1) * NT])
                                        for v4 in range(4):
                                            nc.tensor.matmul(avs[v4][:],
                                                             vt[:, v4 * 128:(v4 + 1) * 128],
                                                             psb[:, ti * NT:(ti + 1) * NT],
                                                             start=(ti == 0), stop=(ti == nkr - 1))
                                if g == 0:
                                    den = awork.tile([1, NT], F32, tag="den_sb")
                                    rec = awork.tile([1, NT], F32, tag="rec")
                                    rec16 = awork.tile([1, NT], BF16, tag="rec16")
                                    nc.vector.tensor_copy(den[:], den_ps[:])
                                    nc.vector.reciprocal_approx_fast(rec[:], den[:])
                                    nc.vector.tensor_copy(rec16[:], rec[:])
                                    rb_ps = apsum.tile([128, NT], F32, tag="pt", bufs=3)
                                    nc.tensor.matmul(rb_ps[:], ones1[:], rec16[:],
                                                     start=True, stop=True)
                                    rd_b = awork.tile([128, NT], BF16, tag="rd_b")
                                    nc.vector.tensor_copy(rd_b[:], rb_ps[:])
                                for v4 in range(4):
                                    vcch = g * 4 + v4
                                    nc.vector.tensor_mul(
                                        attnT_sb[:, vcch * RO + qcol:vcch * RO + qcol + NT],
                                        avs[v4][:], rd_b[:])

                        SCHED = {0: [0, 4], 1: [0, 1, 4, 5],
                                 2: [0, 1, 2, 4, 5, 6], 3: [0, 1, 2, 3, 4, 5, 6, 7]}
                        MOFF = {0: 0, 1: 1024, 2: 3072, 3: 6144}
                        for qi in range(4):
                            attention(qi, SCHED[qi], MOFF[qi])

                      # ---------------- Phase 4: project + residual ----------------
                      with tc.tile_pool(name="pr_stream", bufs=4) as prstream, \
                           tc.tile_pool(name="pr_psum", bufs=4, space="PSUM") as prpsum:
                          for rt in range(RO // 128):
                              for dc in range(2):
                                  ops = prpsum.tile([128, NT], F32, tag="out")
                                  for cch in range(16):
                                      if cch < 8:
                                          lhsT = localT_sb[:, cch * RO + rt * 128:cch * RO + (rt + 1) * 128]
                                      else:
                                          lhsT = attnT_sb[:, (cch - 8) * RO + rt * 128:(cch - 8) * RO + (rt + 1) * 128]
                                      nc.tensor.matmul(ops[:], lhsT,
                                                       proj_sb[:, cch * D + dc * NT:cch * D + (dc + 1) * NT],
                                                       start=(cch == 0), stop=(cch == 15))
                                  xo_t = prstream.tile([128, NT], F32, tag="xo")
                                  nc.sync.dma_start(xo_t[:], xo[rt * 128:(rt + 1) * 128, dc * NT:(dc + 1) * NT])
                                  ot = prstream.tile([128, NT], F32, tag="ot")
                                  nc.vector.tensor_add(ot[:], ops[:], xo_t[:])
                                  nc.sync.dma_start(out[rt * 128:(rt + 1) * 128, dc * NT:(dc + 1) * NT], ot[:])

    nc.compile()
    return nc


_ORDERS = {0: [0, 3, 4, 7, 1, 2, 5, 6], 1: [1, 2, 5, 6, 0, 3, 4, 7]}


def _sigmoid(x):
    return np.where(x >= 0, 1.0 / (1.0 + np.exp(-np.abs(x))),
                    np.exp(-np.abs(x)) / (1.0 + np.exp(-np.abs(x))))


def _prep_inputs(x, expand, project, pbm):
    """Build per-core input maps (host-side sharding)."""
    bf16 = ml_dtypes.bfloat16
    sc = 1.0 / math.sqrt(QK)
    wq = (expand[:, :QK] * sc)
    wk = expand[:, QK:2 * QK]
    lin = expand[:, 2 * QK:2 * QK + E]
    gel = expand[:, 2 * QK + E:]
    wkv = np.concatenate([wk, lin[:, D:], gel[:, D:]], axis=1).astype(bf16)
    wql = np.concatenate([wq, lin[:, :D], gel[:, :D]], axis=1).astype(bf16)
    wproj = project.astype(bf16)

    in_maps = []
    SCHED = {0: [0, 4], 1: [0, 1, 4, 5], 2: [0, 1, 2, 4, 5, 6], 3: [0, 1, 2, 3, 4, 5, 6, 7]}
    NBQ = 512
    for c in range(8):
        b, half = c // 2, c % 2
        order = _ORDERS[half]
        xb = x[b]
        xperm = np.concatenate([xb[blk * NBQ:(blk + 1) * NBQ] for blk in order[:4]], axis=0)
        xt = np.ascontiguousarray(xperm.T).astype(bf16)          # [1024, 2048] own rows only
        xo = np.ascontiguousarray(xperm).astype(np.float32)
        gq_all = np.concatenate([np.arange(blk * NBQ, (blk + 1) * NBQ) for blk in order[:4]]).astype(np.float64)
        # kv slots in FIXED pair order: [A blocks 0,3,4,7 | B blocks 1,2,5,6]
        kv_order = _ORDERS[0][:4] + _ORDERS[1][:4]
        gk_all = np.concatenate([np.arange(blk * NBQ, (blk + 1) * NBQ) for blk in kv_order]).astype(np.float64)

        def expM(gk_sub, gq_sub):
            diff = gk_sub[:, None] - gq_sub[None, :]
            m = np.where(diff <= 0, np.exp(_sigmoid(diff + pbm)), 0.0)
            return m.astype(bf16)

        parts = []
        for qi in range(4):
            gq = gq_all[qi * NBQ:(qi + 1) * NBQ]
            gk = np.concatenate([gk_all[s0 * NBQ:(s0 + 1) * NBQ] for s0 in SCHED[qi]])
            parts.append(expM(gk, gq))
        mskc = np.ascontiguousarray(np.concatenate(parts, axis=0))  # [10240, 512]
        in_maps.append({
            "xt": xt, "xo": xo, "wkv": wkv, "wql": wql, "wproj": wproj,
            "msk": mskc,
        })
    return in_maps


def kernel(x, expand, project, position_bias_mult):
    global LAST_RESULTS
    x = np.asarray(x, dtype=np.float32)
    expand = np.asarray(expand, dtype=np.float32)
    project = np.asarray(project, dtype=np.float32)
    pbm = float(np.asarray(position_bias_mult))

    in_maps = _prep_inputs(x, expand, project, pbm)
    nc = _build_nc()
    res = run_bass_kernel_spmd(nc, in_maps, core_ids=list(range(8)))
    LAST_RESULTS = res

    full = np.empty((B, N, D), dtype=np.float32)
    for c in range(8):
        b, half = c // 2, c % 2
        order = _ORDERS[half]
        o = res.results[c]["out"]
        for qi in range(4):
            blk = order[qi]
            full[b, blk * 512:(blk + 1) * 512] = o[qi * 512:(qi + 1) * 512]
    return full



# revision 4
# speedup vs baseline: 1.3559x; 1.3559x over previous
"""Trainium2 Bass kernel for nn_AttentionLayer (B=4, N=4096, D=1024).

Reference computation:
  nx = layernorm(x)
  h  = nx @ expand                       # [B,N,4352]
  q  = h[:, :128] ; k = h[:, 128:256]
  linear = h[:, 256:2304]; pre_gelu = h[:, 2304:4352]
  gated  = linear * gelu(pre_gelu)       # exact erf gelu
  local  = gated[:, :1024]; v = gated[:, 1024:2048]
  mask[i,j] = j<=i ? sigmoid((j-i)+pbm) : -inf
  attn = softmax(q k^T / sqrt(128) + mask) @ v
  out  = x + concat([local, attn]) @ project

Sharding (8 cores, SPMD): batch b -> core pair (2b, 2b+1).  Per pair,
512-row query blocks interleave for causal load balance: even core owns
blocks {0,3,4,7}, odd owns {1,2,5,6}.  Each core computes LN + expand for
its OWN 2048 rows only; k/v of the other half arrive via pairwise
AllGathers (one k + one v collective per 512-row chunk, issued as each
chunk lands in HBM so the wire time pipelines under the remaining
expand).  The kv slot order is the fixed pair order
[even-core blocks | odd-core blocks], the same on both cores, so the
SPMD attention schedule is uniform: q-slot i attends a fixed slot set
(2/4/6/8 slots).  Causality + position bias use a host-precomputed
multiplicative mask expM = causal ? exp(sigmoid(j-i+pbm)) : 0, but the
mask is only loaded/applied on the 11 (q-slot, kv-slot) pairs where it
differs from 1.0 on either core of the pair (diagonal blocks, band
precursors, and fully-future blocks); elsewhere expM == 1 to bf16
precision because sigmoid(j-i+pbm) underflows ~16 columns past the
diagonal.  P = exp(qk)*expM is normalized by its row sum (no max
subtraction: logits are O(1) after layernorm + xavier weights).

Precision: fp8(e4m3) DoubleRow matmuls (2x PE throughput) for the
expand, attention*V and project matmuls, with per-tensor power-of-2
scales folded into psum-evacuation constants; qk^T stays bf16.  psum
accumulation is f32 throughout.  Measured end-to-end rel err ~1.2e-2
(tolerance 2e-2).
"""

import math

import numpy as np
import ml_dtypes

import concourse.bass as bass
import concourse.mybir as mybir
from concourse import bacc
import concourse.tile as tile
from concourse.bass_utils import run_bass_kernel_spmd

BF16 = mybir.dt.bfloat16
F32 = mybir.dt.float32
FP8 = mybir.dt.float8e4
AF = mybir.ActivationFunctionType
PM = mybir.MatmulPerfMode

B, N, D = 4, 4096, 1024
QK = 128
E = 2048
R = N              # kv rows per core
RO = 2048          # own query rows per core
DCH = D // 128     # 8 contraction chunks
NT = 512           # matmul free-dim tile
W2 = 2176          # 128 (q or k) + 1024 (linear) + 1024 (gelu) cols

# fp8 scales (powers of 2; relative precision is scale-free, these just
# center the dynamic range away from subnormals/overflow)
SX = 16.0          # nx (post-LN activations)
SW = 256.0         # wk / wlin / wgel columns
SWQ = 4096.0       # wq columns (also absorbs the 1/sqrt(qk) prescale)
SV = 8.0           # v / local / attn (the project stationary operands)
SWP = 256.0        # wproj
LN4 = math.log(4.0)  # exp bias => P scaled by 4
USK = 1.0 / (SX * SW)    # 2^-12: k / linear / gelu psum evacuation
USQ = 1.0 / (SX * SWQ)   # 2^-16: q psum evacuation
USO = 1.0 / (SV * SWP)   # 2^-11: project psum evacuation
# NOTE: attnT = av_psum/den_psum exactly: av = sum (4P)(8v) = 32*sum(Pv),
# den = 4*sum(P), so av/den = 8*attn = SV*attn as required.

# attention schedule: q-slot qi attends kv slots 0..qi and 4..4+qi
SCHED = {0: [0, 4], 1: [0, 1, 4, 5], 2: [0, 1, 2, 4, 5, 6], 3: [0, 1, 2, 3, 4, 5, 6, 7]}
# (qi, slot) pairs whose mask differs from all-ones on either core of the
# pair: diagonal blocks, band precursors (j >= i-16 tail), future blocks
MASKED = {0: (0, 4), 1: (1, 4, 5), 2: (1, 2, 6), 3: (3, 6, 7)}
MIDX = {}
for _qi in range(4):
    for _s in MASKED[_qi]:
        MIDX[(_qi, _s)] = len(MIDX)
NMSK = len(MIDX)   # 11

LAST_RESULTS = None  # set by kernel(); test harness reads exec_time_ns


def _build_nc():
    nc = bacc.Bacc(None)

    xt = nc.declare_dram_parameter("xt", [D, RO], BF16, isOutput=False)
    xo = nc.declare_dram_parameter("xo", [RO, D], F32, isOutput=False)
    wkv = nc.declare_dram_parameter("wkv", [D, W2], FP8, isOutput=False)
    wql = nc.declare_dram_parameter("wql", [D, W2], FP8, isOutput=False)
    wproj = nc.declare_dram_parameter("wproj", [E, D], FP8, isOutput=False)
    msk = nc.declare_dram_parameter("msk", [NMSK * 512, NT], BF16, isOutput=False)
    out = nc.declare_dram_parameter("out", [RO, D], F32, isOutput=True)

    with tile.TileContext(nc) as tc:
        with tc.tile_pool(name="const", bufs=1) as cpool:
            ones128 = cpool.tile([128, 1], BF16)
            nc.vector.memset(ones128[:], 1.0)
            ones1 = cpool.tile([1, 128], BF16)
            nc.vector.memset(ones1[:], 1.0)
            ln4b = cpool.tile([128, 1], F32)
            nc.vector.memset(ln4b[:], LN4)

            with tc.tile_pool(name="dram", bufs=1, space="DRAM") as dpool:
                kv_own = [dpool.tile([128, 4, 1024], FP8, name=f"kv_own_{r}")
                          for r in range(4)]
                kv_all = [dpool.tile([2 * 128, 4, 1024], FP8, name=f"kv_all_{r}")
                          for r in range(4)]
                k_own = [dpool.tile([128, NT], BF16, name=f"k_own_{r}")
                         for r in range(4)]
                k_all = [dpool.tile([2 * 128, NT], BF16, name=f"k_all_{r}")
                         for r in range(4)]

                with tc.tile_pool(name="persist", bufs=1) as ppool:
                    kT_sb = ppool.tile([128, R], BF16)         # k^T, hT layout
                    qT_sb = ppool.tile([128, RO], BF16)        # q^T
                    localT_sb = ppool.tile([128, 8, RO], FP8)  # [lc][128, 2048] *SV

                    # ---------------- Phase 1+2: expand ----------------
                    with tc.tile_pool(name="wkv_p", bufs=1) as wkvp, \
                         tc.tile_pool(name="wql_p", bufs=1) as wqlp, \
                         tc.tile_pool(name="ex_stream", bufs=4) as estream, \
                         tc.tile_pool(name="ex_work", bufs=3) as ework, \
                         tc.tile_pool(name="st_work", bufs=2) as swork, \
                         tc.tile_pool(name="ex_psum", bufs=5, space="PSUM") as epsum, \
                         tc.tile_pool(name="st_psum", bufs=2, space="PSUM") as spsum:
                        wkv_sb = wkvp.tile([128, DCH, W2], FP8)
                        for dch in range(DCH):
                            nc.sync.dma_start(wkv_sb[:, dch, :],
                                              wkv[dch * 128:(dch + 1) * 128, :])
                        wql_sb = wqlp.tile([128, DCH, W2], FP8)
                        for dch in range(DCH):
                            nc.sync.dma_start(wql_sb[:, dch, :],
                                              wql[dch * 128:(dch + 1) * 128, :])

                        def stats_chain(rch):
                            """DMA x^T tiles for rch and compute LN scale/shift
                            broadcast tiles.  Emitted one iteration ahead so the
                            DVE math hides under the previous chunk's expand."""
                            rs = rch * NT
                            xts = []
                            for dch in range(DCH):
                                t = estream.tile([128, NT], BF16, tag="xt_e", bufs=10,
                                                 name=f"xt_{rch}_{dch}")
                                nc.sync.dma_start(t[:], xt[dch * 128:(dch + 1) * 128, rs:rs + NT])
                                xts.append(t)
                            mu_ps = spsum.tile([1, NT], F32, tag="stat", name=f"mu_ps_{rch}")
                            sq_ps = spsum.tile([1, NT], F32, tag="stat", name=f"sq_ps_{rch}")
                            # accumulate the 8 d-chunks on DVE (bf16 2x mode), then a
                            # single partition-sum matmul per stat instead of 8 each
                            acc_mu = estream.tile([128, NT], BF16, tag="acc_mu", bufs=3,
                                                  name=f"accmu_{rch}")
                            acc_sq = estream.tile([128, NT], BF16, tag="acc_sq", bufs=3,
                                                  name=f"accsq_{rch}")
                            sq_prev = estream.tile([128, NT], BF16, tag="sq_s", bufs=3,
                                                    name=f"sq_{rch}_0")
                            nc.scalar.activation(sq_prev[:], xts[0][:], AF.Square)
                            nc.vector.tensor_add(acc_mu[:], xts[0][:], xts[1][:])
                            for dch in range(1, DCH):
                                sqt = estream.tile([128, NT], BF16, tag="sq_s", bufs=3,
                                                   name=f"sq_{rch}_{dch}")
                                nc.scalar.activation(sqt[:], xts[dch][:], AF.Square)
                                if dch == 1:
                                    nc.vector.tensor_add(acc_sq[:], sq_prev[:], sqt[:])
                                else:
                                    nc.vector.tensor_add(acc_sq[:], acc_sq[:], sqt[:])
                                if dch >= 2:
                                    nc.vector.tensor_add(acc_mu[:], acc_mu[:], xts[dch][:])
                            nc.tensor.matmul(mu_ps[:], ones128[:], acc_mu[:],
                                             start=True, stop=True)
                            nc.tensor.matmul(sq_ps[:], ones128[:], acc_sq[:],
                                             start=True, stop=True)
                            mu = swork.tile([1, NT], F32, tag="st_mu", bufs=1, name=f"mu_{rch}")
                            e2 = swork.tile([1, NT], F32, tag="st_e2", bufs=1, name=f"e2_{rch}")
                            scr = swork.tile([1, NT], F32, tag="st_scr", bufs=1, name=f"scr_{rch}")
                            nc.vector.tensor_scalar_mul(mu[:], mu_ps[:], 1.0 / D)
                            nc.vector.tensor_scalar_mul(e2[:], sq_ps[:], 1.0 / D)
                            nc.vector.tensor_mul(scr[:], mu[:], mu[:])
                            nc.vector.tensor_sub(e2[:], e2[:], scr[:])
                            nc.vector.tensor_scalar_add(e2[:], e2[:], 1e-5)
                            nc.scalar.activation(e2[:], e2[:], AF.Sqrt)
                            nc.vector.reciprocal_approx_fast(scr[:], e2[:])          # rstd
                            # fold the fp8 activation scale SX into both LN factors
                            nc.vector.scalar_tensor_tensor(
                                mu[:], mu[:], -SX, scr[:],
                                op0=mybir.AluOpType.mult, op1=mybir.AluOpType.mult)  # -SX*mu*rstd
                            rstd16 = swork.tile([1, NT], BF16, tag="st_r16", name=f"r16_{rch}")
                            sneg16 = swork.tile([1, NT], BF16, tag="st_s16", name=f"s16_{rch}")
                            nc.vector.tensor_scalar_mul(rstd16[:], scr[:], SX)
                            nc.vector.tensor_copy(sneg16[:], mu[:])
                            return xts, rstd16, sneg16

                        def bcast_chain(rch, rstd16, sneg16):
                            # rank-1 broadcast [1,NT] -> [128,NT]; emitted mid-expand
                            # of the previous chunk so the DVE math above is hidden
                            bps = spsum.tile([128, NT], F32, tag="bcast", bufs=1, name=f"bps_{rch}")
                            nc.tensor.matmul(bps[:], ones1[:], rstd16[:], start=True, stop=True)
                            rstd_bt = swork.tile([128, NT], BF16, tag="rbt", bufs=3,
                                                 name=f"rbt_{rch}")
                            nc.vector.tensor_copy(rstd_bt[:], bps[:])
                            bps2 = spsum.tile([128, NT], F32, tag="bcast", bufs=1, name=f"bps2_{rch}")
                            nc.tensor.matmul(bps2[:], ones1[:], sneg16[:], start=True, stop=True)
                            sneg_bt = swork.tile([128, NT], BF16, tag="sbt", bufs=3,
                                                 name=f"sbt_{rch}")
                            nc.vector.tensor_copy(sneg_bt[:], bps2[:])
                            return rstd_bt, sneg_bt

                        def center_chain(rch, xts, rstd_bt, sneg_bt):
                            # xpp = SX*(x*rstd - mu*rstd) in fp8, [128, DCH, NT];
                            # emitted mid-way through the PREVIOUS chunk's expand
                            xpp = estream.tile([128, DCH, NT], FP8, tag="xpp", bufs=5,
                                               name=f"xpp_{rch}")
                            for dch in range(DCH):
                                xc = ework.tile([128, NT], BF16, tag="cen", bufs=3,
                                                name=f"cen_{rch}_{dch}")
                                nc.vector.tensor_mul(xc[:], xts[dch][:], rstd_bt[:])
                                nc.vector.tensor_add(xpp[:, dch, :], xc[:], sneg_bt[:])
                            return xpp

                        def v_group(rch, xpp, ms):
                            for m in ms:
                                vlin = ework.tile([128, E // 2], BF16, tag="vlin")
                                vgel = ework.tile([128, E // 2], BF16, tag="vgel")
                                for vc in range(4):
                                    vps = epsum.tile([128, NT], F32, tag="mm")
                                    if vc < 2:
                                        woff = 128 + vc * NT
                                    else:
                                        woff = 1152 + (vc - 2) * NT
                                    for dp in range(DCH // 2):
                                        nc.tensor.matmul(
                                            vps[:],
                                            xpp[:, 2 * dp:2 * dp + 2, m * 128:(m + 1) * 128],
                                            wkv_sb[:, 2 * dp:2 * dp + 2, woff:woff + NT],
                                            start=(dp == 0), stop=(dp == DCH // 2 - 1),
                                            perf_mode=PM.DoubleRow)
                                    if vc < 2:
                                        nc.vector.tensor_scalar_mul(
                                            vlin[:, vc * NT:(vc + 1) * NT], vps[:], USK)
                                    else:
                                        nc.scalar.activation(vgel[:, (vc - 2) * NT:(vc - 1) * NT],
                                                             vps[:], AF.Gelu, scale=USK)
                                vv = ework.tile([128, E // 2], FP8, tag="vv")
                                nc.vector.scalar_tensor_tensor(
                                    vv[:], vlin[:], SV, vgel[:],
                                    op0=mybir.AluOpType.mult, op1=mybir.AluOpType.mult)
                                nc.sync.dma_start(kv_own[rch][:, m, :], vv[:])

                        rch_order = [0, 1, 2, 3]

                        st0 = stats_chain(rch_order[0])
                        bt0 = bcast_chain(rch_order[0], st0[1], st0[2])
                        xpp_stash = center_chain(rch_order[0], st0[0], bt0[0], bt0[1])
                        xpps = {}
                        NOWN = 4
                        for oi, rch in enumerate(rch_order):
                            xpp = xpp_stash
                            xpps[rch] = xpp
                            if oi + 1 < NOWN:
                                nxt = stats_chain(rch_order[oi + 1])
                            # k^T own (hT layout) -> DRAM bounce for the AllGather
                            kps = epsum.tile([128, NT], F32, tag="mm")
                            for dp in range(DCH // 2):
                                nc.tensor.matmul(kps[:],
                                                 wkv_sb[:, 2 * dp:2 * dp + 2, 0:128],
                                                 xpp[:, 2 * dp:2 * dp + 2, :],
                                                 start=(dp == 0), stop=(dp == DCH // 2 - 1),
                                                 perf_mode=PM.DoubleRow)
                            kout = ework.tile([128, NT], BF16, tag="kout", bufs=2,
                                              name=f"kout_{rch}")
                            nc.vector.tensor_scalar_mul(kout[:], kps[:], USK)
                            nc.sync.dma_start(k_own[rch][:], kout[:])
                            v_group(rch, xpp, (0, 1))
                            if oi + 1 < NOWN:
                                nbt = bcast_chain(rch_order[oi + 1], nxt[1], nxt[2])
                                xpp_stash = center_chain(rch_order[oi + 1], nxt[0],
                                                         nbt[0], nbt[1])
                            v_group(rch, xpp, (2, 3))
                            # AllGathers for this chunk's k and v: issued as soon as
                            # the chunk is in HBM so the wire time pipelines under
                            # the remaining expand
                            nc.gpsimd.collective_compute(
                                "AllGather",
                                mybir.AluOpType.bypass,
                                replica_groups=[[0, 1], [2, 3], [4, 5], [6, 7]],
                                ins=[k_own[rch].opt()],
                                outs=[k_all[rch].opt()],
                            )
                            nc.gpsimd.collective_compute(
                                "AllGather",
                                mybir.AluOpType.bypass,
                                replica_groups=[[0, 1], [2, 3], [4, 5], [6, 7]],
                                ins=[kv_own[rch].opt()],
                                outs=[kv_all[rch].opt()],
                            )
                            # kv slot order is [rank0 chunks | rank1 chunks]
                            nc.sync.dma_start(kT_sb[:, rch * NT:(rch + 1) * NT],
                                              k_all[rch][0:128, :])
                            nc.sync.dma_start(kT_sb[:, (4 + rch) * NT:(5 + rch) * NT],
                                              k_all[rch][128:256, :])
                        # loop2: q + local expand (covers the AllGather latency)
                        for rch in rch_order:
                            rs = rch * NT
                            xpp = xpps[rch]
                            qps = epsum.tile([128, NT], F32, tag="mm")
                            for dp in range(DCH // 2):
                                nc.tensor.matmul(qps[:],
                                                 wql_sb[:, 2 * dp:2 * dp + 2, 0:128],
                                                 xpp[:, 2 * dp:2 * dp + 2, :],
                                                 start=(dp == 0), stop=(dp == DCH // 2 - 1),
                                                 perf_mode=PM.DoubleRow)
                            nc.vector.tensor_scalar_mul(qT_sb[:, rs:rs + NT], qps[:], USQ)
                            for lc in range(8):
                                lps = epsum.tile([128, NT], F32, tag="mm")
                                gps = epsum.tile([128, NT], F32, tag="mm")
                                for dp in range(DCH // 2):
                                    nc.tensor.matmul(
                                        lps[:],
                                        wql_sb[:, 2 * dp:2 * dp + 2,
                                               128 + lc * 128:256 + lc * 128],
                                        xpp[:, 2 * dp:2 * dp + 2, :],
                                        start=(dp == 0), stop=(dp == DCH // 2 - 1),
                                        perf_mode=PM.DoubleRow)
                                for dp in range(DCH // 2):
                                    nc.tensor.matmul(
                                        gps[:],
                                        wql_sb[:, 2 * dp:2 * dp + 2,
                                               1152 + lc * 128:1280 + lc * 128],
                                        xpp[:, 2 * dp:2 * dp + 2, :],
                                        start=(dp == 0), stop=(dp == DCH // 2 - 1),
                                        perf_mode=PM.DoubleRow)
                                lgel = ework.tile([128, NT], BF16, tag="lgel")
                                nc.scalar.activation(lgel[:], gps[:], AF.Gelu, scale=USK)
                                llin = ework.tile([128, NT], BF16, tag="llin")
                                nc.vector.tensor_scalar_mul(llin[:], lps[:], USK)
                                nc.vector.scalar_tensor_tensor(
                                    localT_sb[:, lc, rs:rs + NT], llin[:], SV, lgel[:],
                                    op0=mybir.AluOpType.mult, op1=mybir.AluOpType.mult)

                    # ---------------- Phase 3: attention ----------------
                    # attnT pool encloses phase 4 too (read by project)
                    with tc.tile_pool(name="attnT_p", bufs=1) as apool:
                      attnT_sb = apool.tile([128, 8, RO], FP8)   # [vc][128, 2048] *SV
                      proj_sb = apool.tile([128, 16, D], FP8)    # prefetched during attn
                      for _cch in range(16):
                          nc.sync.dma_start(proj_sb[:, _cch, :],
                                            wproj[_cch * 128:(_cch + 1) * 128, :])
                      with tc.tile_pool(name="psb_p", bufs=1) as psbp, \
                         tc.tile_pool(name="at_stream", bufs=6) as astream, \
                         tc.tile_pool(name="at_work", bufs=2) as awork, \
                         tc.tile_pool(name="at_psum", bufs=4, space="PSUM") as apsum, \
                         tc.tile_pool(name="av_psum", bufs=2, space="PSUM") as avpsum:

                        def attention(qi, kr_slots):
                            qcol = qi * NT
                            nkr = len(kr_slots) * 4  # 128-row kr chunks
                            psb = psbp.tile([128, nkr, NT], FP8, tag="psb",
                                            name=f"psb_{qi}")
                            den_ps = apsum.tile([1, NT], F32, tag="den", bufs=1)
                            den_acc = awork.tile([128, NT], BF16, tag="den_acc", bufs=2,
                                                 name=f"den_acc_{qcol}")
                            for i, krs in enumerate(kr_slots):
                                for j in range(4):
                                    ti = i * 4 + j
                                    kr0 = krs * NT + j * 128
                                    pt_ps = apsum.tile([128, NT], F32, tag="pt", bufs=3)
                                    nc.tensor.matmul(pt_ps[:], kT_sb[:, kr0:kr0 + 128],
                                                     qT_sb[:, qcol:qcol + NT],
                                                     start=True, stop=True)
                                    if (qi, krs) in MIDX:
                                        mr0 = MIDX[(qi, krs)] * NT + j * 128
                                        pe = awork.tile([128, NT], BF16, tag="pe", bufs=4)
                                        nc.scalar.activation(pe[:], pt_ps[:], AF.Exp,
                                                             bias=ln4b[:])
                                        mt = astream.tile([128, NT], BF16, tag="mt", bufs=8)
                                        nc.sync.dma_start(mt[:], msk[mr0:mr0 + 128, :])
                                        nc.vector.tensor_mul(psb[:, ti, :], pe[:], mt[:])
                                    else:
                                        nc.scalar.activation(psb[:, ti, :], pt_ps[:],
                                                             AF.Exp, bias=ln4b[:])
                                    if ti == 0:
                                        nc.vector.tensor_copy(den_acc[:], psb[:, 0, :])
                                    else:
                                        nc.vector.tensor_add(den_acc[:], den_acc[:],
                                                             psb[:, ti, :])
                            nc.tensor.matmul(den_ps[:], ones128[:], den_acc[:],
                                             start=True, stop=True)
                            # AV first: the denom reciprocal chain is emitted
                            # after the first AV group so it hides under PE work.
                            rd_b = None
                            for g in range(2):
                                avs = [avpsum.tile([128, NT], F32, tag="av", bufs=4,
                                                   name=f"av{g}_{_i}")
                                       for _i in range(4)]
                                for i, krs in enumerate(kr_slots):
                                    for jp in range(2):
                                        ti0 = i * 4 + 2 * jp
                                        gslot, rb0 = krs % 4, 2 * jp
                                        vrank = krs // 4
                                        vt2 = astream.tile([128, 2, NT], FP8, tag="vt",
                                                           bufs=8)
                                        nc.sync.dma_start(
                                            vt2[:],
                                            kv_all[gslot][vrank * 128:(vrank + 1) * 128,
                                                          rb0:rb0 + 2,
                                                          g * NT:(g + 1) * NT])
                                        for v4 in range(4):
                                            nc.tensor.matmul(
                                                avs[v4][:],
                                                vt2[:, :, v4 * 128:(v4 + 1) * 128],
                                                psb[:, ti0:ti0 + 2, :],
                                                start=(ti0 == 0), stop=(ti0 == nkr - 2),
                                                perf_mode=PM.DoubleRow)
                                if g == 0:
                                    den = awork.tile([1, NT], F32, tag="den_sb")
                                    rec = awork.tile([1, NT], F32, tag="rec")
                                    rec16 = awork.tile([1, NT], BF16, tag="rec16")
                                    nc.vector.tensor_copy(den[:], den_ps[:])
                                    nc.vector.reciprocal_approx_fast(rec[:], den[:])
                                    nc.vector.tensor_copy(rec16[:], rec[:])
                                    rb_ps = apsum.tile([128, NT], F32, tag="pt", bufs=3)
                                    nc.tensor.matmul(rb_ps[:], ones1[:], rec16[:],
                                                     start=True, stop=True)
                                    rd_b = awork.tile([128, NT], BF16, tag="rd_b")
                                    nc.vector.tensor_copy(rd_b[:], rb_ps[:])
                                for v4 in range(4):
                                    nc.vector.tensor_mul(
                                        attnT_sb[:, g * 4 + v4, qcol:qcol + NT],
                                        avs[v4][:], rd_b[:])

                        for qi in range(4):
                            attention(qi, SCHED[qi])

                      # ---------------- Phase 4: project + residual ----------------
                      with tc.tile_pool(name="pr_stream", bufs=4) as prstream, \
                           tc.tile_pool(name="pr_psum", bufs=4, space="PSUM") as prpsum:
                          for rt in range(RO // 128):
                              for dc in range(2):
                                  ops = prpsum.tile([128, NT], F32, tag="out")
                                  for lp in range(4):
                                      nc.tensor.matmul(
                                          ops[:],
                                          localT_sb[:, 2 * lp:2 * lp + 2,
                                                    rt * 128:(rt + 1) * 128],
                                          proj_sb[:, 2 * lp:2 * lp + 2,
                                                  dc * NT:(dc + 1) * NT],
                                          start=(lp == 0), stop=False,
                                          perf_mode=PM.DoubleRow)
                                  for ap_ in range(4):
                                      nc.tensor.matmul(
                                          ops[:],
                                          attnT_sb[:, 2 * ap_:2 * ap_ + 2,
                                                   rt * 128:(rt + 1) * 128],
                                          proj_sb[:, 8 + 2 * ap_:10 + 2 * ap_,
                                                  dc * NT:(dc + 1) * NT],
                                          start=False, stop=(ap_ == 3),
                                          perf_mode=PM.DoubleRow)
                                  xo_t = prstream.tile([128, NT], F32, tag="xo")
                                  nc.sync.dma_start(xo_t[:], xo[rt * 128:(rt + 1) * 128, dc * NT:(dc + 1) * NT])
                                  ot = prstream.tile([128, NT], F32, tag="ot")
                                  nc.vector.scalar_tensor_tensor(
                                      ot[:], ops[:], USO, xo_t[:],
                                      op0=mybir.AluOpType.mult, op1=mybir.AluOpType.add)
                                  nc.sync.dma_start(out[rt * 128:(rt + 1) * 128, dc * NT:(dc + 1) * NT], ot[:])

    nc.compile()
    return nc


_ORDERS = {0: [0, 3, 4, 7, 1, 2, 5, 6], 1: [1, 2, 5, 6, 0, 3, 4, 7]}


def _sigmoid(x):
    return np.where(x >= 0, 1.0 / (1.0 + np.exp(-np.abs(x))),
                    np.exp(-np.abs(x)) / (1.0 + np.exp(-np.abs(x))))


def _prep_inputs(x, expand, project, pbm):
    """Build per-core input maps (host-side sharding)."""
    bf16 = ml_dtypes.bfloat16
    fp8 = ml_dtypes.float8_e4m3
    sc = 1.0 / math.sqrt(QK)
    wq = expand[:, :QK] * (sc * SWQ)
    wk = expand[:, QK:2 * QK] * SW
    lin = expand[:, 2 * QK:2 * QK + E] * SW
    gel = expand[:, 2 * QK + E:] * SW
    wkv = np.concatenate([wk, lin[:, D:], gel[:, D:]], axis=1).astype(fp8)
    wql = np.concatenate([wq, lin[:, :D], gel[:, :D]], axis=1).astype(fp8)
    wproj = (project * SWP).astype(fp8)

    in_maps = []
    NBQ = 512
    for c in range(8):
        b, half = c // 2, c % 2
        order = _ORDERS[half]
        xb = x[b]
        xperm = np.concatenate([xb[blk * NBQ:(blk + 1) * NBQ] for blk in order[:4]], axis=0)
        xt = np.ascontiguousarray(xperm.T).astype(bf16)          # [1024, 2048] own rows only
        xo = np.ascontiguousarray(xperm).astype(np.float32)
        # kv slots in FIXED pair order: [A blocks 0,3,4,7 | B blocks 1,2,5,6]
        kv_order = _ORDERS[0][:4] + _ORDERS[1][:4]

        def expM(gk_sub, gq_sub):
            diff = gk_sub[:, None] - gq_sub[None, :]
            m = np.where(diff <= 0, np.exp(_sigmoid(diff + pbm)), 0.0)
            return m.astype(bf16)

        parts = []
        for qi in range(4):
            gq = np.arange(order[qi] * NBQ, (order[qi] + 1) * NBQ).astype(np.float64)
            for s in MASKED[qi]:
                gblk = kv_order[s]
                gk = np.arange(gblk * NBQ, (gblk + 1) * NBQ).astype(np.float64)
                parts.append(expM(gk, gq))
        mskc = np.ascontiguousarray(np.concatenate(parts, axis=0))  # [NMSK*512, 512]
        in_maps.append({
            "xt": xt, "xo": xo, "wkv": wkv, "wql": wql, "wproj": wproj,
            "msk": mskc,
        })
    return in_maps


def kernel(x, expand, project, position_bias_mult):
    global LAST_RESULTS
    x = np.asarray(x, dtype=np.float32)
    expand = np.asarray(expand, dtype=np.float32)
    project = np.asarray(project, dtype=np.float32)
    pbm = float(np.asarray(position_bias_mult))

    in_maps = _prep_inputs(x, expand, project, pbm)
    nc = _build_nc()
    res = run_bass_kernel_spmd(nc, in_maps, core_ids=list(range(8)))
    LAST_RESULTS = res

    full = np.empty((B, N, D), dtype=np.float32)
    for c in range(8):
        b, half = c // 2, c % 2
        order = _ORDERS[half]
        o = res.results[c]["out"]
        for qi in range(4):
            blk = order[qi]
            full[b, blk * 512:(blk + 1) * 512] = o[qi * 512:(qi + 1) * 512]
    return full


# revision 5
# speedup vs baseline: 1.5471x; 1.1410x over previous
"""Trainium2 Bass kernel for nn_AttentionLayer (B=4, N=4096, D=1024).

Reference computation:
  nx = layernorm(x)
  h  = nx @ expand                       # [B,N,4352]
  q  = h[:, :128] ; k = h[:, 128:256]
  linear = h[:, 256:2304]; pre_gelu = h[:, 2304:4352]
  gated  = linear * gelu(pre_gelu)       # exact erf gelu
  local  = gated[:, :1024]; v = gated[:, 1024:2048]
  mask[i,j] = j<=i ? sigmoid((j-i)+pbm) : -inf
  attn = softmax(q k^T / sqrt(128) + mask) @ v
  out  = x + concat([local, attn]) @ project

Sharding (8 cores, SPMD): batch b -> core pair (2b, 2b+1).  Per pair,
512-row query blocks interleave for causal load balance: even core owns
blocks {0,3,4,7}, odd owns {1,2,5,6}.  Each core computes LN + expand for
its OWN 2048 rows only; k/v of the other half arrive via pairwise
AllGathers (one k + one v collective per 512-row chunk, issued as each
chunk lands in HBM so the wire time pipelines under the remaining
expand).  The kv slot order is the fixed pair order
[even-core blocks | odd-core blocks], the same on both cores, so the
SPMD attention schedule is uniform: q-slot i attends a fixed slot set
(2/4/6/8 slots).  Causality + position bias use a host-precomputed
multiplicative mask expM = causal ? exp(sigmoid(j-i+pbm)) : 0, but the
mask is only loaded/applied on the 11 (q-slot, kv-slot) pairs where it
differs from 1.0 on either core of the pair (diagonal blocks, band
precursors, and fully-future blocks); elsewhere expM == 1 to bf16
precision because sigmoid(j-i+pbm) underflows ~16 columns past the
diagonal.  P = exp(qk)*expM is normalized by its row sum (no max
subtraction: logits are O(1) after layernorm + xavier weights).

Precision: fp8(e4m3) DoubleRow matmuls (2x PE throughput) for the
expand, attention*V and project matmuls, with per-tensor power-of-2
scales folded into psum-evacuation constants; qk^T stays bf16.  psum
accumulation is f32 throughout.  Measured end-to-end rel err ~1.2e-2
(tolerance 2e-2).

Schedule notes: all x^T tiles are preloaded up front (no per-chunk DMA
dependency chains); DMAs are issued as single multi-dim descriptors
(host pre-transposes arrays into [128, chunk, cols] layouts) to keep
the sync-engine descriptor-issue time off the critical path; psum
evacuations that feed a multiply are fused into one
scalar_tensor_tensor; the attention denominator matmul is emitted after
the first AV psum group so the PE never waits on the DVE add chain.
"""

import math

import numpy as np
import ml_dtypes

import concourse.bass as bass
import concourse.mybir as mybir
from concourse import bacc
import concourse.tile as tile
from concourse.bass_utils import run_bass_kernel_spmd

BF16 = mybir.dt.bfloat16
F32 = mybir.dt.float32
FP8 = mybir.dt.float8e4
AF = mybir.ActivationFunctionType
PM = mybir.MatmulPerfMode
MUL = mybir.AluOpType.mult
ADD = mybir.AluOpType.add

B, N, D = 4, 4096, 1024
QK = 128
E = 2048
R = N              # kv rows per core
RO = 2048          # own query rows per core
DCH = D // 128     # 8 contraction chunks
NT = 512           # matmul free-dim tile
W2 = 2176          # 128 (q or k) + 1024 (linear) + 1024 (gelu) cols

# fp8 scales (powers of 2; relative precision is scale-free, these just
# center the dynamic range away from subnormals/overflow)
SX = 16.0          # nx (post-LN activations)
SW = 256.0         # wk / wlin / wgel columns
SWQ = 4096.0       # wq columns (also absorbs the 1/sqrt(qk) prescale)
SV = 8.0           # v / local / attn (the project stationary operands)
SWP = 256.0        # wproj
LN4 = math.log(4.0)  # exp bias => P scaled by 4
USK = 1.0 / (SX * SW)    # 2^-12: k / linear / gelu psum evacuation
USQ = 1.0 / (SX * SWQ)   # 2^-16: q psum evacuation
USO = 1.0 / (SV * SWP)   # 2^-11: project psum evacuation
# NOTE: attnT = av_psum/den_psum exactly: av = sum (4P)(8v) = 32*sum(Pv),
# den = 4*sum(P), so av/den = 8*attn = SV*attn as required.

# attention schedule: q-slot qi attends kv slots 0..qi and 4..4+qi
SCHED = {0: [0, 4], 1: [0, 1, 4, 5], 2: [0, 1, 2, 4, 5, 6], 3: [0, 1, 2, 3, 4, 5, 6, 7]}
# (qi, slot) pairs whose mask differs from all-ones on either core of the
# pair: diagonal blocks, band precursors (j >= i-16 tail), future blocks
MASKED = {0: (0, 4), 1: (1, 4, 5), 2: (1, 2, 6), 3: (3, 6, 7)}
MIDX = {}
for _qi in range(4):
    for _s in MASKED[_qi]:
        MIDX[(_qi, _s)] = len(MIDX)
NMSK = len(MIDX)   # 11

LAST_RESULTS = None  # set by kernel(); test harness reads exec_time_ns


def _build_nc():
    nc = bacc.Bacc(None)

    # host pre-transposed layouts: leading dim is the SBUF partition
    xt = nc.declare_dram_parameter("xt", [128, DCH, RO], BF16, isOutput=False)
    xo = nc.declare_dram_parameter("xo", [RO, D], BF16, isOutput=False)
    wkv = nc.declare_dram_parameter("wkv", [128, DCH, W2], FP8, isOutput=False)
    wql = nc.declare_dram_parameter("wql", [128, DCH, W2], FP8, isOutput=False)
    wproj = nc.declare_dram_parameter("wproj", [128, 16, D], FP8, isOutput=False)
    msk = nc.declare_dram_parameter("msk", [128, NMSK, 4, NT], BF16, isOutput=False)
    out = nc.declare_dram_parameter("out", [RO, D], F32, isOutput=True)

    with tile.TileContext(nc) as tc:
        with tc.tile_pool(name="const", bufs=1) as cpool:
            ones128 = cpool.tile([128, 1], BF16)
            nc.vector.memset(ones128[:], 1.0)
            ones1 = cpool.tile([1, 128], BF16)
            nc.vector.memset(ones1[:], 1.0)
            ln4b = cpool.tile([128, 1], F32)
            nc.vector.memset(ln4b[:], LN4)

            with tc.tile_pool(name="dram", bufs=1, space="DRAM") as dpool:
                kv_own = [dpool.tile([128, 4, 1024], FP8, name=f"kv_own_{r}")
                          for r in range(4)]
                kv_all = [dpool.tile([2 * 128, 4, 1024], FP8, name=f"kv_all_{r}")
                          for r in range(4)]
                k_own = [dpool.tile([128, NT], BF16, name=f"k_own_{r}")
                         for r in range(4)]
                k_all = [dpool.tile([2 * 128, NT], BF16, name=f"k_all_{r}")
                         for r in range(4)]

                with tc.tile_pool(name="persist", bufs=1) as ppool, \
                     tc.tile_pool(name="attnT_p", bufs=1) as apool:
                    kT_sb = ppool.tile([128, R], BF16)         # k^T, hT layout
                    qT_sb = ppool.tile([128, RO], BF16)        # q^T
                    localT_sb = ppool.tile([128, 8, RO], FP8)  # [lc][128, 2048] *SV
                    attnT_sb = apool.tile([128, 8, RO], FP8)   # [vc][128, 2048] *SV
                    proj_sb = apool.tile([128, 16, D], FP8)    # DMA'd at loop2 start

                    # ---------------- Phase 1+2: expand ----------------
                    with tc.tile_pool(name="xt_p", bufs=1) as xtp, \
                         tc.tile_pool(name="wkv_p", bufs=1) as wkvp, \
                         tc.tile_pool(name="wql_p", bufs=1) as wqlp, \
                         tc.tile_pool(name="ex_stream", bufs=4) as estream, \
                         tc.tile_pool(name="ex_work", bufs=3) as ework, \
                         tc.tile_pool(name="st_work", bufs=2) as swork, \
                         tc.tile_pool(name="ex_psum", bufs=5, space="PSUM") as epsum, \
                         tc.tile_pool(name="st_psum", bufs=2, space="PSUM") as spsum:
                        # preload ALL x^T tiles + weights up front; chunk 0's x
                        # first (feeds the first stats chain), then wkv (first
                        # matmuls), remaining x, then wql (needed only in loop2)
                        xt_all = [xtp.tile([128, DCH, NT], BF16, name=f"xt_{r}")
                                  for r in range(4)]
                        wkv_sb = wkvp.tile([128, DCH, W2], FP8)
                        wql_sb = wqlp.tile([128, DCH, W2], FP8)
                        for h in range(2):
                            nc.sync.dma_start(xt_all[0][:, 4 * h:4 * h + 4, :],
                                              xt[:, 4 * h:4 * h + 4, 0:NT])
                        for dq in range(4):
                            nc.sync.dma_start(wkv_sb[:, 2 * dq:2 * dq + 2, :],
                                              wkv[:, 2 * dq:2 * dq + 2, :])
                        for r in range(1, 4):
                            for h in range(2):
                                nc.sync.dma_start(
                                    xt_all[r][:, 4 * h:4 * h + 4, :],
                                    xt[:, 4 * h:4 * h + 4, r * NT:(r + 1) * NT])
                        for dq in range(4):
                            nc.sync.dma_start(wql_sb[:, 2 * dq:2 * dq + 2, :],
                                              wql[:, 2 * dq:2 * dq + 2, :])

                        def stats_chain(rch):
                            """LN scale/shift for chunk rch from the preloaded
                            x^T tiles.  Emitted one iteration ahead so the DVE
                            math hides under the previous chunk's expand."""
                            xts = xt_all[rch]
                            mu_ps = spsum.tile([1, NT], F32, tag="stat", name=f"mu_ps_{rch}")
                            sq_ps = spsum.tile([1, NT], F32, tag="stat", name=f"sq_ps_{rch}")
                            # accumulate the 8 d-chunks on DVE (bf16 2x mode), then a
                            # single partition-sum matmul per stat instead of 8 each
                            acc_mu = estream.tile([128, NT], BF16, tag="acc_mu", bufs=3,
                                                  name=f"accmu_{rch}")
                            acc_sq = estream.tile([128, NT], BF16, tag="acc_sq", bufs=3,
                                                  name=f"accsq_{rch}")
                            sq_prev = estream.tile([128, NT], BF16, tag="sq_s", bufs=3,
                                                    name=f"sq_{rch}_0")
                            nc.scalar.activation(sq_prev[:], xts[:, 0, :], AF.Square)
                            nc.vector.tensor_add(acc_mu[:], xts[:, 0, :], xts[:, 1, :])
                            for dch in range(1, DCH):
                                sqt = estream.tile([128, NT], BF16, tag="sq_s", bufs=3,
                                                   name=f"sq_{rch}_{dch}")
                                nc.scalar.activation(sqt[:], xts[:, dch, :], AF.Square)
                                if dch == 1:
                                    nc.vector.tensor_add(acc_sq[:], sq_prev[:], sqt[:])
                                else:
                                    nc.vector.tensor_add(acc_sq[:], acc_sq[:], sqt[:])
                                if dch >= 2:
                                    nc.vector.tensor_add(acc_mu[:], acc_mu[:], xts[:, dch, :])
                            nc.tensor.matmul(mu_ps[:], ones128[:], acc_mu[:],
                                             start=True, stop=True)
                            nc.tensor.matmul(sq_ps[:], ones128[:], acc_sq[:],
                                             start=True, stop=True)
                            mu = swork.tile([1, NT], F32, tag="st_mu", bufs=1, name=f"mu_{rch}")
                            e2 = swork.tile([1, NT], F32, tag="st_e2", bufs=1, name=f"e2_{rch}")
                            scr = swork.tile([1, NT], F32, tag="st_scr", bufs=1, name=f"scr_{rch}")
                            nc.vector.tensor_scalar_mul(mu[:], mu_ps[:], 1.0 / D)
                            nc.vector.tensor_scalar_mul(e2[:], sq_ps[:], 1.0 / D)
                            nc.vector.tensor_mul(scr[:], mu[:], mu[:])
                            nc.vector.tensor_sub(e2[:], e2[:], scr[:])
                            nc.vector.tensor_scalar_add(e2[:], e2[:], 1e-5)
                            nc.scalar.activation(e2[:], e2[:], AF.Sqrt)
                            nc.vector.reciprocal_approx_fast(scr[:], e2[:])          # rstd
                            # fold the fp8 activation scale SX into both LN factors
                            nc.vector.scalar_tensor_tensor(
                                mu[:], mu[:], -SX, scr[:], op0=MUL, op1=MUL)  # -SX*mu*rstd
                            rstd16 = swork.tile([1, NT], BF16, tag="st_r16", name=f"r16_{rch}")
                            sneg16 = swork.tile([1, NT], BF16, tag="st_s16", name=f"s16_{rch}")
                            nc.vector.tensor_scalar_mul(rstd16[:], scr[:], SX)
                            nc.vector.tensor_copy(sneg16[:], mu[:])
                            return rstd16, sneg16

                        def bcast_chain(rch, rstd16, sneg16):
                            # rank-1 broadcast [1,NT] -> [128,NT]; emitted mid-expand
                            # of the previous chunk so the DVE math above is hidden
                            bps = spsum.tile([128, NT], F32, tag="bcast", bufs=1, name=f"bps_{rch}")
                            nc.tensor.matmul(bps[:], ones1[:], rstd16[:], start=True, stop=True)
                            rstd_bt = swork.tile([128, NT], BF16, tag="rbt", bufs=3,
                                                 name=f"rbt_{rch}")
                            nc.vector.tensor_copy(rstd_bt[:], bps[:])
                            bps2 = spsum.tile([128, NT], F32, tag="bcast", bufs=1, name=f"bps2_{rch}")
                            nc.tensor.matmul(bps2[:], ones1[:], sneg16[:], start=True, stop=True)
                            sneg_bt = swork.tile([128, NT], BF16, tag="sbt", bufs=3,
                                                 name=f"sbt_{rch}")
                            nc.vector.tensor_copy(sneg_bt[:], bps2[:])
                            return rstd_bt, sneg_bt

                        def center_chain(rch, rstd_bt, sneg_bt):
                            # xpp = SX*(x*rstd - mu*rstd) in fp8, [128, DCH, NT];
                            # emitted mid-way through the PREVIOUS chunk's expand
                            xts = xt_all[rch]
                            xpp = estream.tile([128, DCH, NT], FP8, tag="xpp", bufs=5,
                                               name=f"xpp_{rch}")
                            for dch in range(DCH):
                                xc = ework.tile([128, NT], BF16, tag="cen", bufs=3,
                                                name=f"cen_{rch}_{dch}")
                                nc.vector.tensor_mul(xc[:], xts[:, dch, :], rstd_bt[:])
                                nc.vector.tensor_add(xpp[:, dch, :], xc[:], sneg_bt[:])
                            return xpp

                        def v_group(rch, xpp, ms):
                            for m in ms:
                                vgel = ework.tile([128, E // 2], BF16, tag="vgel")
                                vv = ework.tile([128, E // 2], FP8, tag="vv")
                                # gelu columns first so the fused lin*gelu stt
                                # has its second operand ready
                                for vc in (2, 3, 0, 1):
                                    vps = epsum.tile([128, NT], F32, tag="mm")
                                    if vc < 2:
                                        woff = 128 + vc * NT
                                    else:
                                        woff = 1152 + (vc - 2) * NT
                                    for dp in range(DCH // 2):
                                        nc.tensor.matmul(
                                            vps[:],
                                            xpp[:, 2 * dp:2 * dp + 2, m * 128:(m + 1) * 128],
                                            wkv_sb[:, 2 * dp:2 * dp + 2, woff:woff + NT],
                                            start=(dp == 0), stop=(dp == DCH // 2 - 1),
                                            perf_mode=PM.DoubleRow)
                                    if vc >= 2:
                                        nc.scalar.activation(vgel[:, (vc - 2) * NT:(vc - 1) * NT],
                                                             vps[:], AF.Gelu, scale=USK)
                                    else:
                                        # vv = (lin_psum * USK*SV) * gelu, fused
                                        nc.vector.scalar_tensor_tensor(
                                            vv[:, vc * NT:(vc + 1) * NT], vps[:],
                                            USK * SV, vgel[:, vc * NT:(vc + 1) * NT],
                                            op0=MUL, op1=MUL)
                                nc.sync.dma_start(kv_own[rch][:, m, :], vv[:])

                        rch_order = [0, 1, 2, 3]

                        st0 = stats_chain(rch_order[0])
                        bt0 = bcast_chain(rch_order[0], st0[0], st0[1])
                        xpp_stash = center_chain(rch_order[0], bt0[0], bt0[1])
                        xpps = {}
                        NOWN = 4
                        for oi, rch in enumerate(rch_order):
                            xpp = xpp_stash
                            xpps[rch] = xpp
                            if oi + 1 < NOWN:
                                nxt = stats_chain(rch_order[oi + 1])
                            # k^T own (hT layout) -> DRAM bounce for the AllGather
                            kps = epsum.tile([128, NT], F32, tag="mm")
                            for dp in range(DCH // 2):
                                nc.tensor.matmul(kps[:],
                                                 wkv_sb[:, 2 * dp:2 * dp + 2, 0:128],
                                                 xpp[:, 2 * dp:2 * dp + 2, :],
                                                 start=(dp == 0), stop=(dp == DCH // 2 - 1),
                                                 perf_mode=PM.DoubleRow)
                            kout = ework.tile([128, NT], BF16, tag="kout", bufs=2,
                                              name=f"kout_{rch}")
                            nc.scalar.activation(kout[:], kps[:], AF.Copy, scale=USK)
                            nc.sync.dma_start(k_own[rch][:], kout[:])
                            v_group(rch, xpp, (0, 1))
                            if oi + 1 < NOWN:
                                nbt = bcast_chain(rch_order[oi + 1], nxt[0], nxt[1])
                                xpp_stash = center_chain(rch_order[oi + 1],
                                                         nbt[0], nbt[1])
                            v_group(rch, xpp, (2, 3))
                            # AllGathers for this chunk's k and v: issued as soon as
                            # the chunk is in HBM so the wire time pipelines under
                            # the remaining expand
                            nc.gpsimd.collective_compute(
                                "AllGather",
                                mybir.AluOpType.bypass,
                                replica_groups=[[0, 1], [2, 3], [4, 5], [6, 7]],
                                ins=[k_own[rch].opt()],
                                outs=[k_all[rch].opt()],
                            )
                            nc.gpsimd.collective_compute(
                                "AllGather",
                                mybir.AluOpType.bypass,
                                replica_groups=[[0, 1], [2, 3], [4, 5], [6, 7]],
                                ins=[kv_own[rch].opt()],
                                outs=[kv_all[rch].opt()],
                            )
                            # kv slot order is [rank0 chunks | rank1 chunks]
                            nc.sync.dma_start(kT_sb[:, rch * NT:(rch + 1) * NT],
                                              k_all[rch][0:128, :])
                            nc.sync.dma_start(kT_sb[:, (4 + rch) * NT:(5 + rch) * NT],
                                              k_all[rch][128:256, :])
                        # loop2: q + local expand (covers the AllGather latency);
                        # also prefetch the project weights now - the sync queue
                        # is quiet here and they're needed right after attention
                        for pq in range(4):
                            nc.sync.dma_start(proj_sb[:, 4 * pq:4 * pq + 4, :],
                                              wproj[:, 4 * pq:4 * pq + 4, :])
                        for rch in rch_order:
                            rs = rch * NT
                            xpp = xpps[rch]
                            qps = epsum.tile([128, NT], F32, tag="mm")
                            for dp in range(DCH // 2):
                                nc.tensor.matmul(qps[:],
                                                 wql_sb[:, 2 * dp:2 * dp + 2, 0:128],
                                                 xpp[:, 2 * dp:2 * dp + 2, :],
                                                 start=(dp == 0), stop=(dp == DCH // 2 - 1),
                                                 perf_mode=PM.DoubleRow)
                            nc.scalar.activation(qT_sb[:, rs:rs + NT], qps[:],
                                                 AF.Copy, scale=USQ)
                            for lc in range(8):
                                lps = epsum.tile([128, NT], F32, tag="mm")
                                gps = epsum.tile([128, NT], F32, tag="mm")
                                for dp in range(DCH // 2):
                                    nc.tensor.matmul(
                                        gps[:],
                                        wql_sb[:, 2 * dp:2 * dp + 2,
                                               1152 + lc * 128:1280 + lc * 128],
                                        xpp[:, 2 * dp:2 * dp + 2, :],
                                        start=(dp == 0), stop=(dp == DCH // 2 - 1),
                                        perf_mode=PM.DoubleRow)
                                for dp in range(DCH // 2):
                                    nc.tensor.matmul(
                                        lps[:],
                                        wql_sb[:, 2 * dp:2 * dp + 2,
                                               128 + lc * 128:256 + lc * 128],
                                        xpp[:, 2 * dp:2 * dp + 2, :],
                                        start=(dp == 0), stop=(dp == DCH // 2 - 1),
                                        perf_mode=PM.DoubleRow)
                                lgel = ework.tile([128, NT], BF16, tag="lgel")
                                nc.scalar.activation(lgel[:], gps[:], AF.Gelu, scale=USK)
                                # localT = (lin_psum * USK*SV) * gelu, fused
                                nc.vector.scalar_tensor_tensor(
                                    localT_sb[:, lc, rs:rs + NT], lps[:], USK * SV,
                                    lgel[:], op0=MUL, op1=MUL)

                    # ---------------- Phase 3: attention ----------------
                    with tc.tile_pool(name="psb_p", bufs=2) as psbp, \
                         tc.tile_pool(name="at_stream", bufs=6) as astream, \
                         tc.tile_pool(name="at_work", bufs=2) as awork, \
                         tc.tile_pool(name="at_psum", bufs=4, space="PSUM") as apsum, \
                         tc.tile_pool(name="av_psum", bufs=2, space="PSUM") as avpsum:

                        def attention(qi, kr_slots):
                            qcol = qi * NT
                            nkr = len(kr_slots) * 4  # 128-row kr chunks
                            psb = psbp.tile([128, nkr, NT], FP8, tag="psb",
                                            name=f"psb_{qi}")
                            den_ps = apsum.tile([1, NT], F32, tag="den", bufs=1)
                            den_acc = awork.tile([128, NT], BF16, tag="den_acc", bufs=2,
                                                 name=f"den_acc_{qcol}")
                            for i, krs in enumerate(kr_slots):
                                mt4 = None
                                if (qi, krs) in MIDX:
                                    mt4 = astream.tile([128, 4, NT], BF16, tag="mt",
                                                       bufs=6)
                                    nc.sync.dma_start(mt4[:],
                                                      msk[:, MIDX[(qi, krs)], :, :])
                                for j in range(4):
                                    ti = i * 4 + j
                                    kr0 = krs * NT + j * 128
                                    pt_ps = apsum.tile([128, NT], F32, tag="pt", bufs=3)
                                    nc.tensor.matmul(pt_ps[:], kT_sb[:, kr0:kr0 + 128],
                                                     qT_sb[:, qcol:qcol + NT],
                                                     start=True, stop=True)
                                    if mt4 is not None:
                                        pe = awork.tile([128, NT], BF16, tag="pe", bufs=4)
                                        nc.scalar.activation(pe[:], pt_ps[:], AF.Exp,
                                                             bias=ln4b[:])
                                        nc.vector.tensor_mul(psb[:, ti, :], pe[:],
                                                             mt4[:, j, :])
                                    else:
                                        nc.scalar.activation(psb[:, ti, :], pt_ps[:],
                                                             AF.Exp, bias=ln4b[:])
                                    if ti == 0:
                                        nc.vector.tensor_copy(den_acc[:], psb[:, 0, :])
                                    else:
                                        nc.vector.tensor_add(den_acc[:], den_acc[:],
                                                             psb[:, ti, :])
                            # AV first: the denominator matmul + reciprocal chain
                            # are emitted after the first AV group so the PE never
                            # waits on the DVE den-add chain.
                            rd_b = None
                            for g in range(2):
                                avs = [avpsum.tile([128, NT], F32, tag="av", bufs=4,
                                                   name=f"av{g}_{_i}")
                                       for _i in range(4)]
                                for i, krs in enumerate(kr_slots):
                                    gslot, vrank = krs % 4, krs // 4
                                    vt4 = astream.tile([128, 4, NT], FP8, tag="vt",
                                                       bufs=6)
                                    nc.sync.dma_start(
                                        vt4[:],
                                        kv_all[gslot][vrank * 128:(vrank + 1) * 128,
                                                      :, g * NT:(g + 1) * NT])
                                    for jp in range(2):
                                        ti0 = i * 4 + 2 * jp
                                        for v4 in range(4):
                                            nc.tensor.matmul(
                                                avs[v4][:],
                                                vt4[:, 2 * jp:2 * jp + 2,
                                                    v4 * 128:(v4 + 1) * 128],
                                                psb[:, ti0:ti0 + 2, :],
                                                start=(ti0 == 0), stop=(ti0 == nkr - 2),
                                                perf_mode=PM.DoubleRow)
                                if g == 0:
                                    nc.tensor.matmul(den_ps[:], ones128[:], den_acc[:],
                                                     start=True, stop=True)
                                    den = awork.tile([1, NT], F32, tag="den_sb")
                                    rec = awork.tile([1, NT], F32, tag="rec")
                                    rec16 = awork.tile([1, NT], BF16, tag="rec16")
                                    nc.vector.tensor_copy(den[:], den_ps[:])
                                    nc.vector.reciprocal_approx_fast(rec[:], den[:])
                                    nc.vector.tensor_copy(rec16[:], rec[:])
                                    rb_ps = apsum.tile([128, NT], F32, tag="pt", bufs=3)
                                    nc.tensor.matmul(rb_ps[:], ones1[:], rec16[:],
                                                     start=True, stop=True)
                                    rd_b = awork.tile([128, NT], BF16, tag="rd_b")
                                    nc.vector.tensor_copy(rd_b[:], rb_ps[:])
                                for v4 in range(4):
                                    nc.vector.tensor_mul(
                                        attnT_sb[:, g * 4 + v4, qcol:qcol + NT],
                                        avs[v4][:], rd_b[:])

                        for qi in range(4):
                            attention(qi, SCHED[qi])

                    # ---------------- Phase 4: project + residual ----------------
                    with tc.tile_pool(name="pr_stream", bufs=4) as prstream, \
                         tc.tile_pool(name="pr_psum", bufs=4, space="PSUM") as prpsum:
                        for rt in range(RO // 128):
                            xo_t = prstream.tile([128, D], BF16, tag="xo")
                            nc.sync.dma_start(xo_t[:], xo[rt * 128:(rt + 1) * 128, :])
                            ot = prstream.tile([128, D], F32, tag="ot")
                            for dc in range(2):
                                ops = prpsum.tile([128, NT], F32, tag="out")
                                for lp in range(4):
                                    nc.tensor.matmul(
                                        ops[:],
                                        localT_sb[:, 2 * lp:2 * lp + 2,
                                                  rt * 128:(rt + 1) * 128],
                                        proj_sb[:, 2 * lp:2 * lp + 2,
                                                dc * NT:(dc + 1) * NT],
                                        start=(lp == 0), stop=False,
                                        perf_mode=PM.DoubleRow)
                                for ap_ in range(4):
                                    nc.tensor.matmul(
                                        ops[:],
                                        attnT_sb[:, 2 * ap_:2 * ap_ + 2,
                                                 rt * 128:(rt + 1) * 128],
                                        proj_sb[:, 8 + 2 * ap_:10 + 2 * ap_,
                                                dc * NT:(dc + 1) * NT],
                                        start=False, stop=(ap_ == 3),
                                        perf_mode=PM.DoubleRow)
                                nc.vector.scalar_tensor_tensor(
                                    ot[:, dc * NT:(dc + 1) * NT], ops[:], USO,
                                    xo_t[:, dc * NT:(dc + 1) * NT],
                                    op0=MUL, op1=ADD)
                            nc.sync.dma_start(out[rt * 128:(rt + 1) * 128, :], ot[:])

    nc.compile()
    return nc


_ORDERS = {0: [0, 3, 4, 7, 1, 2, 5, 6], 1: [1, 2, 5, 6, 0, 3, 4, 7]}


def _sigmoid(x):
    return np.where(x >= 0, 1.0 / (1.0 + np.exp(-np.abs(x))),
                    np.exp(-np.abs(x)) / (1.0 + np.exp(-np.abs(x))))


def _chunk_part(a, nch):
    """[nch*128, C] -> [128, nch, C] with [p, i, c] = a[i*128+p, c]."""
    return np.ascontiguousarray(
        a.reshape(nch, 128, a.shape[1]).transpose(1, 0, 2))


def _prep_inputs(x, expand, project, pbm):
    """Build per-core input maps (host-side sharding)."""
    bf16 = ml_dtypes.bfloat16
    fp8 = ml_dtypes.float8_e4m3
    sc = 1.0 / math.sqrt(QK)
    wq = expand[:, :QK] * (sc * SWQ)
    wk = expand[:, QK:2 * QK] * SW
    lin = expand[:, 2 * QK:2 * QK + E] * SW
    gel = expand[:, 2 * QK + E:] * SW
    wkv = _chunk_part(
        np.concatenate([wk, lin[:, D:], gel[:, D:]], axis=1), DCH).astype(fp8)
    wql = _chunk_part(
        np.concatenate([wq, lin[:, :D], gel[:, :D]], axis=1), DCH).astype(fp8)
    wproj = _chunk_part(project * SWP, 16).astype(fp8)

    in_maps = []
    NBQ = 512
    for c in range(8):
        b, half = c // 2, c % 2
        order = _ORDERS[half]
        xb = x[b]
        xperm = np.concatenate([xb[blk * NBQ:(blk + 1) * NBQ] for blk in order[:4]], axis=0)
        xt = _chunk_part(np.ascontiguousarray(xperm.T), DCH).astype(bf16)  # [128, DCH, 2048]
        xo = np.ascontiguousarray(xperm).astype(bf16)
        # kv slots in FIXED pair order: [A blocks 0,3,4,7 | B blocks 1,2,5,6]
        kv_order = _ORDERS[0][:4] + _ORDERS[1][:4]

        def expM(gk_sub, gq_sub):
            diff = gk_sub[:, None] - gq_sub[None, :]
            m = np.where(diff <= 0, np.exp(_sigmoid(diff + pbm)), 0.0)
            return m.astype(np.float32)

        parts = []
        for qi in range(4):
            gq = np.arange(order[qi] * NBQ, (order[qi] + 1) * NBQ).astype(np.float64)
            for s in MASKED[qi]:
                gblk = kv_order[s]
                gk = np.arange(gblk * NBQ, (gblk + 1) * NBQ).astype(np.float64)
                # [512, 512] -> [128, 4, 512]
                parts.append(_chunk_part(expM(gk, gq), 4)[:, None, :, :])
        mskc = np.concatenate(parts, axis=1).astype(bf16)  # [128, NMSK, 4, 512]
        in_maps.append({
            "xt": xt, "xo": xo, "wkv": wkv, "wql": wql, "wproj": wproj,
            "msk": np.ascontiguousarray(mskc),
        })
    return in_maps


def kernel(x, expand, project, position_bias_mult):
    global LAST_RESULTS
    x = np.asarray(x, dtype=np.float32)
    expand = np.asarray(expand, dtype=np.float32)
    project = np.asarray(project, dtype=np.float32)
    pbm = float(np.asarray(position_bias_mult))

    in_maps = _prep_inputs(x, expand, project, pbm)
    nc = _build_nc()
    res = run_bass_kernel_spmd(nc, in_maps, core_ids=list(range(8)))
    LAST_RESULTS = res

    full = np.empty((B, N, D), dtype=np.float32)
    for c in range(8):
        b, half = c // 2, c % 2
        order = _ORDERS[half]
        o = res.results[c]["out"]
        for qi in range(4):
            blk = order[qi]
            full[b, blk * 512:(blk + 1) * 512] = o[qi * 512:(qi + 1) * 512]
    return full


# revision 7
# speedup vs baseline: 1.6285x; 1.0526x over previous
"""Trainium2 Bass kernel for nn_AttentionLayer (B=4, N=4096, D=1024).

Reference computation:
  nx = layernorm(x)
  h  = nx @ expand                       # [B,N,4352]
  q  = h[:, :128] ; k = h[:, 128:256]
  linear = h[:, 256:2304]; pre_gelu = h[:, 2304:4352]
  gated  = linear * gelu(pre_gelu)       # exact erf gelu
  local  = gated[:, :1024]; v = gated[:, 1024:2048]
  mask[i,j] = j<=i ? sigmoid((j-i)+pbm) : -inf
  attn = softmax(q k^T / sqrt(128) + mask) @ v
  out  = x + concat([local, attn]) @ project

Sharding (8 cores, SPMD): batch b -> core pair (2b, 2b+1).  Per pair,
512-row query blocks interleave for causal load balance: even core owns
blocks {0,3,4,7}, odd owns {1,2,5,6}.  Each core computes LN + expand for
its OWN 2048 rows only; k/v of the other half arrive via pairwise
AllGathers (one k + one v collective per 512-row chunk, issued as each
chunk lands in HBM so the wire time pipelines under the remaining
expand).  The kv slot order is the fixed pair order
[even-core blocks | odd-core blocks], the same on both cores, so the
SPMD attention schedule is uniform: q-slot i attends a fixed slot set
(2/4/6/8 slots).  Causality + position bias use a host-precomputed
multiplicative mask expM = causal ? exp(sigmoid(j-i+pbm)) : 0, but the
mask is only loaded/applied on the 11 (q-slot, kv-slot) pairs where it
differs from 1.0 on either core of the pair (diagonal blocks, band
precursors, and fully-future blocks); elsewhere expM == 1 to bf16
precision because sigmoid(j-i+pbm) underflows ~16 columns past the
diagonal.  P = exp(qk)*expM is normalized by its row sum (no max
subtraction: logits are O(1) after layernorm + xavier weights).

Precision: fp8(e4m3) DoubleRow matmuls (2x PE throughput) for the
expand, attention*V and project matmuls, with per-tensor power-of-2
scales folded into psum-evacuation constants; qk^T stays bf16.  psum
accumulation is f32 throughout.  Measured end-to-end rel err ~1.2e-2
(tolerance 2e-2).

Schedule notes: all x^T tiles are preloaded up front (no per-chunk DMA
dependency chains); DMAs are issued as single multi-dim descriptors
(host pre-transposes arrays into [128, chunk, cols] layouts) to keep
the sync-engine descriptor-issue time off the critical path; psum
evacuations that feed a multiply are fused into one
scalar_tensor_tensor; the attention denominator matmul is emitted after
the first AV psum group so the PE never waits on the DVE add chain.
"""

import math

import numpy as np
import ml_dtypes

import concourse.bass as bass
import concourse.mybir as mybir
from concourse import bacc
import concourse.tile as tile
from concourse.bass_utils import run_bass_kernel_spmd

BF16 = mybir.dt.bfloat16
F32 = mybir.dt.float32
FP8 = mybir.dt.float8e4
AF = mybir.ActivationFunctionType
PM = mybir.MatmulPerfMode
MUL = mybir.AluOpType.mult
ADD = mybir.AluOpType.add

B, N, D = 4, 4096, 1024
QK = 128
E = 2048
R = N              # kv rows per core
RO = 2048          # own query rows per core
DCH = D // 128     # 8 contraction chunks
NT = 512           # matmul free-dim tile
W2 = 2176          # 128 (q or k) + 1024 (linear) + 1024 (gelu) cols

# fp8 scales (powers of 2; relative precision is scale-free, these just
# center the dynamic range away from subnormals/overflow)
SX = 16.0          # nx (post-LN activations)
SW = 256.0         # wk / wlin / wgel columns
SWQ = 4096.0       # wq columns (also absorbs the 1/sqrt(qk) prescale)
SV = 8.0           # v / local / attn (the project stationary operands)
SWP = 256.0        # wproj
LN4 = math.log(4.0)  # exp bias => P scaled by 4
USK = 1.0 / (SX * SW)    # 2^-12: k / linear / gelu psum evacuation
USQ = 1.0 / (SX * SWQ)   # 2^-16: q psum evacuation
USO = 1.0 / (SV * SWP)   # 2^-11: project psum evacuation
# NOTE: attnT = av_psum/den_psum exactly: av = sum (4P)(8v) = 32*sum(Pv),
# den = 4*sum(P), so av/den = 8*attn = SV*attn as required.

# attention schedule: q-slot qi attends kv slots 0..qi and 4..4+qi
SCHED = {0: [0, 4], 1: [0, 1, 4, 5], 2: [0, 1, 2, 4, 5, 6], 3: [0, 1, 2, 3, 4, 5, 6, 7]}
# (qi, slot) pairs whose mask differs from all-ones on either core of the
# pair: diagonal blocks, band precursors (j >= i-16 tail), future blocks
MASKED = {0: (0, 4), 1: (1, 4, 5), 2: (1, 2, 6), 3: (3, 6, 7)}
MIDX = {}
for _qi in range(4):
    for _s in MASKED[_qi]:
        MIDX[(_qi, _s)] = len(MIDX)
NMSK = len(MIDX)   # 11

LAST_RESULTS = None  # set by kernel(); test harness reads exec_time_ns


def _build_nc():
    nc = bacc.Bacc(None)

    # host pre-transposed layouts: leading dim is the SBUF partition
    xt = nc.declare_dram_parameter("xt", [128, DCH, RO], BF16, isOutput=False)
    xo = nc.declare_dram_parameter("xo", [RO, D], BF16, isOutput=False)
    wkv = nc.declare_dram_parameter("wkv", [128, DCH, W2], FP8, isOutput=False)
    wql = nc.declare_dram_parameter("wql", [128, DCH, W2], FP8, isOutput=False)
    wproj = nc.declare_dram_parameter("wproj", [128, 16, D], FP8, isOutput=False)
    msk = nc.declare_dram_parameter("msk", [128, NMSK, 4, NT], BF16, isOutput=False)
    out = nc.declare_dram_parameter("out", [RO, D], F32, isOutput=True)

    with tile.TileContext(nc) as tc:
        with tc.tile_pool(name="const", bufs=1) as cpool:
            ones128 = cpool.tile([128, 1], BF16)
            nc.vector.memset(ones128[:], 1.0)
            ones1 = cpool.tile([1, 128], BF16)
            nc.vector.memset(ones1[:], 1.0)
            ln4b = cpool.tile([128, 1], F32)
            nc.vector.memset(ln4b[:], LN4)

            with tc.tile_pool(name="dram", bufs=1, space="DRAM") as dpool:
                kv_own = [dpool.tile([128, 4, 1024], FP8, name=f"kv_own_{r}")
                          for r in range(4)]
                kv_all = [dpool.tile([2 * 128, 4, 1024], FP8, name=f"kv_all_{r}")
                          for r in range(4)]
                k_own = [dpool.tile([128, NT], BF16, name=f"k_own_{r}")
                         for r in range(4)]
                k_all = [dpool.tile([2 * 128, NT], BF16, name=f"k_all_{r}")
                         for r in range(4)]

                with tc.tile_pool(name="persist", bufs=1) as ppool, \
                     tc.tile_pool(name="attnT_p", bufs=1) as apool:
                    kT_sb = ppool.tile([128, R], BF16)         # k^T, hT layout
                    qT_sb = ppool.tile([128, RO], BF16)        # q^T
                    localT_sb = ppool.tile([128, 8, RO], FP8)  # [lc][128, 2048] *SV
                    attnT_sb = apool.tile([128, 8, RO], FP8)   # [vc][128, 2048] *SV
                    proj_sb = apool.tile([128, 16, D], FP8)    # DMA'd at loop2 start

                    # ---------------- Phase 1+2: expand ----------------
                    with tc.tile_pool(name="xt_p", bufs=1) as xtp, \
                         tc.tile_pool(name="wkv_p", bufs=1) as wkvp, \
                         tc.tile_pool(name="wql_p", bufs=1) as wqlp, \
                         tc.tile_pool(name="ex_stream", bufs=4) as estream, \
                         tc.tile_pool(name="ex_work", bufs=3) as ework, \
                         tc.tile_pool(name="st_work", bufs=2) as swork, \
                         tc.tile_pool(name="ex_psum", bufs=5, space="PSUM") as epsum, \
                         tc.tile_pool(name="st_psum", bufs=2, space="PSUM") as spsum:
                        # preload ALL x^T tiles + weights up front; chunk 0's x
                        # first (feeds the first stats chain), then wkv (first
                        # matmuls), remaining x, then wql (needed only in loop2)
                        xt_all = [xtp.tile([128, DCH, NT], BF16, name=f"xt_{r}")
                                  for r in range(4)]
                        wkv_sb = wkvp.tile([128, DCH, W2], FP8)
                        wql_sb = wqlp.tile([128, DCH, W2], FP8)
                        for h in range(2):
                            nc.sync.dma_start(xt_all[0][:, 4 * h:4 * h + 4, :],
                                              xt[:, 4 * h:4 * h + 4, 0:NT])
                        for dq in range(4):
                            nc.sync.dma_start(wkv_sb[:, 2 * dq:2 * dq + 2, :],
                                              wkv[:, 2 * dq:2 * dq + 2, :])
                        for r in range(1, 4):
                            for h in range(2):
                                nc.sync.dma_start(
                                    xt_all[r][:, 4 * h:4 * h + 4, :],
                                    xt[:, 4 * h:4 * h + 4, r * NT:(r + 1) * NT])
                        for dq in range(4):
                            nc.sync.dma_start(wql_sb[:, 2 * dq:2 * dq + 2, :],
                                              wql[:, 2 * dq:2 * dq + 2, :])

                        def stats_chain(rch):
                            """LN scale/shift for chunk rch from the preloaded
                            x^T tiles.  Emitted one iteration ahead so the DVE
                            math hides under the previous chunk's expand."""
                            xts = xt_all[rch]
                            mu_ps = spsum.tile([1, NT], F32, tag="stat", name=f"mu_ps_{rch}")
                            sq_ps = spsum.tile([1, NT], F32, tag="stat", name=f"sq_ps_{rch}")
                            # accumulate the 8 d-chunks on DVE (bf16 2x mode), then a
                            # single partition-sum matmul per stat instead of 8 each
                            acc_mu = estream.tile([128, NT], BF16, tag="acc_mu", bufs=3,
                                                  name=f"accmu_{rch}")
                            acc_sq = estream.tile([128, NT], BF16, tag="acc_sq", bufs=3,
                                                  name=f"accsq_{rch}")
                            sq_prev = estream.tile([128, NT], BF16, tag="sq_s", bufs=3,
                                                    name=f"sq_{rch}_0")
                            nc.scalar.activation(sq_prev[:], xts[:, 0, :], AF.Square)
                            nc.vector.tensor_add(acc_mu[:], xts[:, 0, :], xts[:, 1, :])
                            for dch in range(1, DCH):
                                sqt = estream.tile([128, NT], BF16, tag="sq_s", bufs=3,
                                                   name=f"sq_{rch}_{dch}")
                                nc.scalar.activation(sqt[:], xts[:, dch, :], AF.Square)
                                if dch == 1:
                                    nc.vector.tensor_add(acc_sq[:], sq_prev[:], sqt[:])
                                else:
                                    nc.vector.tensor_add(acc_sq[:], acc_sq[:], sqt[:])
                                if dch >= 2:
                                    nc.vector.tensor_add(acc_mu[:], acc_mu[:], xts[:, dch, :])
                            nc.tensor.matmul(mu_ps[:], ones128[:], acc_mu[:],
                                             start=True, stop=True)
                            nc.tensor.matmul(sq_ps[:], ones128[:], acc_sq[:],
                                             start=True, stop=True)
                            mu = swork.tile([1, NT], F32, tag="st_mu", bufs=1, name=f"mu_{rch}")
                            e2 = swork.tile([1, NT], F32, tag="st_e2", bufs=1, name=f"e2_{rch}")
                            scr = swork.tile([1, NT], F32, tag="st_scr", bufs=1, name=f"scr_{rch}")
                            nc.vector.tensor_scalar_mul(mu[:], mu_ps[:], 1.0 / D)
                            nc.vector.tensor_scalar_mul(e2[:], sq_ps[:], 1.0 / D)
                            nc.vector.tensor_mul(scr[:], mu[:], mu[:])
                            nc.vector.tensor_sub(e2[:], e2[:], scr[:])
                            nc.vector.tensor_scalar_add(e2[:], e2[:], 1e-5)
                            nc.scalar.activation(e2[:], e2[:], AF.Sqrt)
                            nc.vector.reciprocal_approx_fast(scr[:], e2[:])          # rstd
                            # fold the fp8 activation scale SX into both LN factors
                            nc.vector.scalar_tensor_tensor(
                                mu[:], mu[:], -SX, scr[:], op0=MUL, op1=MUL)  # -SX*mu*rstd
                            rstd16 = swork.tile([1, NT], BF16, tag="st_r16", name=f"r16_{rch}")
                            sneg16 = swork.tile([1, NT], BF16, tag="st_s16", name=f"s16_{rch}")
                            nc.vector.tensor_scalar_mul(rstd16[:], scr[:], SX)
                            nc.vector.tensor_copy(sneg16[:], mu[:])
                            return rstd16, sneg16

                        def bcast_chain(rch, rstd16, sneg16):
                            # rank-1 broadcast [1,NT] -> [128,NT]; emitted mid-expand
                            # of the previous chunk so the DVE math above is hidden
                            bps = spsum.tile([128, NT], F32, tag="bcast", bufs=1, name=f"bps_{rch}")
                            nc.tensor.matmul(bps[:], ones1[:], rstd16[:], start=True, stop=True)
                            rstd_bt = swork.tile([128, NT], BF16, tag="rbt", bufs=3,
                                                 name=f"rbt_{rch}")
                            nc.vector.tensor_copy(rstd_bt[:], bps[:])
                            bps2 = spsum.tile([128, NT], F32, tag="bcast", bufs=1, name=f"bps2_{rch}")
                            nc.tensor.matmul(bps2[:], ones1[:], sneg16[:], start=True, stop=True)
                            sneg_bt = swork.tile([128, NT], BF16, tag="sbt", bufs=3,
                                                 name=f"sbt_{rch}")
                            nc.vector.tensor_copy(sneg_bt[:], bps2[:])
                            return rstd_bt, sneg_bt

                        def center_chain(rch, rstd_bt, sneg_bt):
                            # xpp = SX*(x*rstd - mu*rstd) in fp8, [128, DCH, NT];
                            # emitted mid-way through the PREVIOUS chunk's expand
                            xts = xt_all[rch]
                            xpp = estream.tile([128, DCH, NT], FP8, tag="xpp", bufs=5,
                                               name=f"xpp_{rch}")
                            for dch in range(DCH):
                                xc = ework.tile([128, NT], BF16, tag="cen", bufs=3,
                                                name=f"cen_{rch}_{dch}")
                                nc.vector.tensor_mul(xc[:], xts[:, dch, :], rstd_bt[:])
                                nc.vector.tensor_add(xpp[:, dch, :], xc[:], sneg_bt[:])
                            return xpp

                        def v_group(rch, xpp, ms):
                            for m in ms:
                                vgel = ework.tile([128, E // 2], BF16, tag="vgel")
                                vv = ework.tile([128, E // 2], FP8, tag="vv")
                                # gelu columns first so the fused lin*gelu stt
                                # has its second operand ready
                                for vc in (2, 3, 0, 1):
                                    vps = epsum.tile([128, NT], F32, tag="mm")
                                    if vc < 2:
                                        woff = 128 + vc * NT
                                    else:
                                        woff = 1152 + (vc - 2) * NT
                                    for dp in range(DCH // 2):
                                        nc.tensor.matmul(
                                            vps[:],
                                            xpp[:, 2 * dp:2 * dp + 2, m * 128:(m + 1) * 128],
                                            wkv_sb[:, 2 * dp:2 * dp + 2, woff:woff + NT],
                                            start=(dp == 0), stop=(dp == DCH // 2 - 1),
                                            perf_mode=PM.DoubleRow)
                                    if vc >= 2:
                                        nc.scalar.activation(vgel[:, (vc - 2) * NT:(vc - 1) * NT],
                                                             vps[:], AF.Gelu, scale=USK)
                                    else:
                                        # vv = (lin_psum * USK*SV) * gelu, fused
                                        nc.vector.scalar_tensor_tensor(
                                            vv[:, vc * NT:(vc + 1) * NT], vps[:],
                                            USK * SV, vgel[:, vc * NT:(vc + 1) * NT],
                                            op0=MUL, op1=MUL)
                                nc.sync.dma_start(kv_own[rch][:, m, :], vv[:])

                        rch_order = [0, 1, 2, 3]

                        st0 = stats_chain(rch_order[0])
                        bt0 = bcast_chain(rch_order[0], st0[0], st0[1])
                        xpp_stash = center_chain(rch_order[0], bt0[0], bt0[1])
                        xpps = {}
                        NOWN = 4
                        for oi, rch in enumerate(rch_order):
                            xpp = xpp_stash
                            xpps[rch] = xpp
                            if oi + 1 < NOWN:
                                nxt = stats_chain(rch_order[oi + 1])
                            # k^T own (hT layout) -> DRAM bounce for the AllGather
                            kps = epsum.tile([128, NT], F32, tag="mm")
                            for dp in range(DCH // 2):
                                nc.tensor.matmul(kps[:],
                                                 wkv_sb[:, 2 * dp:2 * dp + 2, 0:128],
                                                 xpp[:, 2 * dp:2 * dp + 2, :],
                                                 start=(dp == 0), stop=(dp == DCH // 2 - 1),
                                                 perf_mode=PM.DoubleRow)
                            kout = ework.tile([128, NT], BF16, tag="kout", bufs=2,
                                              name=f"kout_{rch}")
                            nc.scalar.activation(kout[:], kps[:], AF.Copy, scale=USK)
                            nc.sync.dma_start(k_own[rch][:], kout[:])
                            v_group(rch, xpp, (0, 1))
                            if oi + 1 < NOWN:
                                nbt = bcast_chain(rch_order[oi + 1], nxt[0], nxt[1])
                                xpp_stash = center_chain(rch_order[oi + 1],
                                                         nbt[0], nbt[1])
                            v_group(rch, xpp, (2, 3))
                            # AllGathers for this chunk's k and v: issued as soon as
                            # the chunk is in HBM so the wire time pipelines under
                            # the remaining expand
                            nc.gpsimd.collective_compute(
                                "AllGather",
                                mybir.AluOpType.bypass,
                                replica_groups=[[0, 1], [2, 3], [4, 5], [6, 7]],
                                ins=[k_own[rch].opt()],
                                outs=[k_all[rch].opt()],
                            )
                            nc.gpsimd.collective_compute(
                                "AllGather",
                                mybir.AluOpType.bypass,
                                replica_groups=[[0, 1], [2, 3], [4, 5], [6, 7]],
                                ins=[kv_own[rch].opt()],
                                outs=[kv_all[rch].opt()],
                            )
                            # kv slot order is [rank0 chunks | rank1 chunks]
                            nc.sync.dma_start(kT_sb[:, rch * NT:(rch + 1) * NT],
                                              k_all[rch][0:128, :])
                            nc.sync.dma_start(kT_sb[:, (4 + rch) * NT:(5 + rch) * NT],
                                              k_all[rch][128:256, :])
                        # loop2: q + local expand (covers the AllGather latency);
                        # also prefetch the project weights now - the sync queue
                        # is quiet here and they're needed right after attention
                        for pq in range(4):
                            nc.sync.dma_start(proj_sb[:, 4 * pq:4 * pq + 4, :],
                                              wproj[:, 4 * pq:4 * pq + 4, :])
                        for rch in rch_order:
                            rs = rch * NT
                            xpp = xpps[rch]
                            qps = epsum.tile([128, NT], F32, tag="mm")
                            for dp in range(DCH // 2):
                                nc.tensor.matmul(qps[:],
                                                 wql_sb[:, 2 * dp:2 * dp + 2, 0:128],
                                                 xpp[:, 2 * dp:2 * dp + 2, :],
                                                 start=(dp == 0), stop=(dp == DCH // 2 - 1),
                                                 perf_mode=PM.DoubleRow)
                            nc.scalar.activation(qT_sb[:, rs:rs + NT], qps[:],
                                                 AF.Copy, scale=USQ)
                            for lc in range(8):
                                lps = epsum.tile([128, NT], F32, tag="mm")
                                gps = epsum.tile([128, NT], F32, tag="mm")
                                for dp in range(DCH // 2):
                                    nc.tensor.matmul(
                                        gps[:],
                                        wql_sb[:, 2 * dp:2 * dp + 2,
                                               1152 + lc * 128:1280 + lc * 128],
                                        xpp[:, 2 * dp:2 * dp + 2, :],
                                        start=(dp == 0), stop=(dp == DCH // 2 - 1),
                                        perf_mode=PM.DoubleRow)
                                for dp in range(DCH // 2):
                                    nc.tensor.matmul(
                                        lps[:],
                                        wql_sb[:, 2 * dp:2 * dp + 2,
                                               128 + lc * 128:256 + lc * 128],
                                        xpp[:, 2 * dp:2 * dp + 2, :],
                                        start=(dp == 0), stop=(dp == DCH // 2 - 1),
                                        perf_mode=PM.DoubleRow)
                                lgel = ework.tile([128, NT], BF16, tag="lgel")
                                nc.scalar.activation(lgel[:], gps[:], AF.Gelu, scale=USK)
                                # localT = (lin_psum * USK*SV) * gelu, fused
                                nc.vector.scalar_tensor_tensor(
                                    localT_sb[:, lc, rs:rs + NT], lps[:], USK * SV,
                                    lgel[:], op0=MUL, op1=MUL)

                    # ---------------- Phase 3: attention ----------------
                    with tc.tile_pool(name="psb_p", bufs=2) as psbp, \
                         tc.tile_pool(name="at_stream", bufs=6) as astream, \
                         tc.tile_pool(name="at_work", bufs=2) as awork, \
                         tc.tile_pool(name="at_psum", bufs=4, space="PSUM") as apsum, \
                         tc.tile_pool(name="av_psum", bufs=2, space="PSUM") as avpsum:

                        def make_producer(qi):
                            """psb production for q-slot qi: per tile one qk
                            matmul + exp (+mask mul) + split den add.  Returned
                            as (state, generator) so consume() can weave single
                            tiles of qi+1's production between its AV matmul
                            groups: the PE then always has AV work while the
                            scalar-engine exp chain paces production."""
                            kr_slots = SCHED[qi]
                            qcol = qi * NT
                            nkr = len(kr_slots) * 4
                            st = {
                                "psb": psbp.tile([128, nkr, NT], FP8, tag="psb",
                                                 name=f"psb_{qi}"),
                                "den_a": awork.tile([128, NT], BF16, tag="den_a",
                                                    bufs=2, name=f"den_a_{qi}"),
                                "den_b": awork.tile([128, NT], BF16, tag="den_b",
                                                    bufs=2, name=f"den_b_{qi}"),
                            }

                            def gen():
                                psb = st["psb"]
                                for i, krs in enumerate(kr_slots):
                                    mt4 = None
                                    if (qi, krs) in MIDX:
                                        mt4 = astream.tile([128, 4, NT], BF16,
                                                           tag="mt", bufs=6)
                                        nc.sync.dma_start(
                                            mt4[:], msk[:, MIDX[(qi, krs)], :, :])
                                    for j in range(4):
                                        ti = i * 4 + j
                                        kr0 = krs * NT + j * 128
                                        pt_ps = apsum.tile([128, NT], F32, tag="pt",
                                                           bufs=3)
                                        nc.tensor.matmul(pt_ps[:],
                                                         kT_sb[:, kr0:kr0 + 128],
                                                         qT_sb[:, qcol:qcol + NT],
                                                         start=True, stop=True)
                                        if mt4 is not None:
                                            pe = awork.tile([128, NT], BF16,
                                                            tag="pe", bufs=4)
                                            nc.scalar.activation(pe[:], pt_ps[:],
                                                                 AF.Exp, bias=ln4b[:])
                                            nc.vector.tensor_mul(psb[:, ti, :], pe[:],
                                                                 mt4[:, j, :])
                                        else:
                                            nc.scalar.activation(psb[:, ti, :],
                                                                 pt_ps[:], AF.Exp,
                                                                 bias=ln4b[:])
                                        acc = st["den_a"] if ti % 2 == 0 else st["den_b"]
                                        if ti < 2:
                                            nc.vector.tensor_copy(acc[:], psb[:, ti, :])
                                        else:
                                            nc.vector.tensor_add(acc[:], acc[:],
                                                                 psb[:, ti, :])
                                        yield
                            return st, gen()

                        def weave(nxt, k):
                            if nxt is None:
                                return
                            for _ in range(k):
                                if next(nxt, "DONE") == "DONE":
                                    break

                        def consume(qi, st, nxt):
                            kr_slots = SCHED[qi]
                            qcol = qi * NT
                            nkr = len(kr_slots) * 4
                            psb = st["psb"]
                            nslots = len(kr_slots)
                            # spread qi+1's production tiles over this qi's AV
                            # slot positions (both g groups + the den gap)
                            nwv = 4 * len(SCHED[qi + 1]) if qi < 3 else 0
                            npos = 2 * nslots
                            kw = -(-max(nwv - 3, 0) // npos) if nwv else 0
                            rd_b = None
                            for g in range(2):
                                avs = [avpsum.tile([128, NT], F32, tag="av", bufs=4,
                                                   name=f"av{g}_{_i}")
                                       for _i in range(4)]
                                for i, krs in enumerate(kr_slots):
                                    gslot, vrank = krs % 4, krs // 4
                                    vt4 = astream.tile([128, 4, NT], FP8, tag="vt",
                                                       bufs=6)
                                    nc.sync.dma_start(
                                        vt4[:],
                                        kv_all[gslot][vrank * 128:(vrank + 1) * 128,
                                                      :, g * NT:(g + 1) * NT])
                                    for jp in range(2):
                                        ti0 = i * 4 + 2 * jp
                                        for v4 in range(4):
                                            nc.tensor.matmul(
                                                avs[v4][:],
                                                vt4[:, 2 * jp:2 * jp + 2,
                                                    v4 * 128:(v4 + 1) * 128],
                                                psb[:, ti0:ti0 + 2, :],
                                                start=(ti0 == 0), stop=(ti0 == nkr - 2),
                                                perf_mode=PM.DoubleRow)
                                    weave(nxt, kw)
                                if g == 0:
                                    den_ps = apsum.tile([1, NT], F32, tag="den",
                                                        bufs=1)
                                    nc.tensor.matmul(den_ps[:], ones128[:],
                                                     st["den_a"][:],
                                                     start=True, stop=False)
                                    nc.tensor.matmul(den_ps[:], ones128[:],
                                                     st["den_b"][:],
                                                     start=False, stop=True)
                                    weave(nxt, 3)
                                    den = awork.tile([1, NT], F32, tag="den_sb")
                                    rec = awork.tile([1, NT], F32, tag="rec")
                                    rec16 = awork.tile([1, NT], BF16, tag="rec16")
                                    nc.vector.tensor_copy(den[:], den_ps[:])
                                    nc.vector.reciprocal_approx_fast(rec[:], den[:])
                                    nc.vector.tensor_copy(rec16[:], rec[:])
                                    rb_ps = apsum.tile([128, NT], F32, tag="pt",
                                                       bufs=3)
                                    nc.tensor.matmul(rb_ps[:], ones1[:], rec16[:],
                                                     start=True, stop=True)
                                    rd_b = awork.tile([128, NT], BF16, tag="rd_b")
                                    nc.vector.tensor_copy(rd_b[:], rb_ps[:])
                                for v4 in range(4):
                                    nc.vector.tensor_mul(
                                        attnT_sb[:, g * 4 + v4, qcol:qcol + NT],
                                        avs[v4][:], rd_b[:])
                            weave(nxt, 99)  # drain any remainder

                        st, pgen = make_producer(0)
                        weave(pgen, 99)
                        for qi in range(4):
                            nxt_st = nxt_gen = None
                            if qi < 3:
                                nxt_st, nxt_gen = make_producer(qi + 1)
                            consume(qi, st, nxt_gen)
                            st = nxt_st

                    # ---------------- Phase 4: project + residual ----------------
                    with tc.tile_pool(name="pr_stream", bufs=4) as prstream, \
                         tc.tile_pool(name="pr_psum", bufs=4, space="PSUM") as prpsum:
                        for rt in range(RO // 128):
                            xo_t = prstream.tile([128, D], BF16, tag="xo")
                            nc.sync.dma_start(xo_t[:], xo[rt * 128:(rt + 1) * 128, :])
                            ot = prstream.tile([128, D], F32, tag="ot")
                            for dc in range(2):
                                ops = prpsum.tile([128, NT], F32, tag="out")
                                for lp in range(4):
                                    nc.tensor.matmul(
                                        ops[:],
                                        localT_sb[:, 2 * lp:2 * lp + 2,
                                                  rt * 128:(rt + 1) * 128],
                                        proj_sb[:, 2 * lp:2 * lp + 2,
                                                dc * NT:(dc + 1) * NT],
                                        start=(lp == 0), stop=False,
                                        perf_mode=PM.DoubleRow)
                                for ap_ in range(4):
                                    nc.tensor.matmul(
                                        ops[:],
                                        attnT_sb[:, 2 * ap_:2 * ap_ + 2,
                                                 rt * 128:(rt + 1) * 128],
                                        proj_sb[:, 8 + 2 * ap_:10 + 2 * ap_,
                                                dc * NT:(dc + 1) * NT],
                                        start=False, stop=(ap_ == 3),
                                        perf_mode=PM.DoubleRow)
                                nc.vector.scalar_tensor_tensor(
                                    ot[:, dc * NT:(dc + 1) * NT], ops[:], USO,
                                    xo_t[:, dc * NT:(dc + 1) * NT],
                                    op0=MUL, op1=ADD)
                                nc.sync.dma_start(
                                    out[rt * 128:(rt + 1) * 128, dc * NT:(dc + 1) * NT],
                                    ot[:, dc * NT:(dc + 1) * NT])

    nc.compile()
    return nc


_ORDERS = {0: [0, 3, 4, 7, 1, 2, 5, 6], 1: [1, 2, 5, 6, 0, 3, 4, 7]}


def _sigmoid(x):
    return np.where(x >= 0, 1.0 / (1.0 + np.exp(-np.abs(x))),
                    np.exp(-np.abs(x)) / (1.0 + np.exp(-np.abs(x))))


def _chunk_part(a, nch):
    """[nch*128, C] -> [128, nch, C] with [p, i, c] = a[i*128+p, c]."""
    return np.ascontiguousarray(
        a.reshape(nch, 128, a.shape[1]).transpose(1, 0, 2))


def _prep_inputs(x, expand, project, pbm):
    """Build per-core input maps (host-side sharding)."""
    bf16 = ml_dtypes.bfloat16
    fp8 = ml_dtypes.float8_e4m3
    sc = 1.0 / math.sqrt(QK)
    wq = expand[:, :QK] * (sc * SWQ)
    wk = expand[:, QK:2 * QK] * SW
    lin = expand[:, 2 * QK:2 * QK + E] * SW
    gel = expand[:, 2 * QK + E:] * SW
    wkv = _chunk_part(
        np.concatenate([wk, lin[:, D:], gel[:, D:]], axis=1), DCH).astype(fp8)
    wql = _chunk_part(
        np.concatenate([wq, lin[:, :D], gel[:, :D]], axis=1), DCH).astype(fp8)
    wproj = _chunk_part(project * SWP, 16).astype(fp8)

    in_maps = []
    NBQ = 512
    for c in range(8):
        b, half = c // 2, c % 2
        order = _ORDERS[half]
        xb = x[b]
        xperm = np.concatenate([xb[blk * NBQ:(blk + 1) * NBQ] for blk in order[:4]], axis=0)
        xt = _chunk_part(np.ascontiguousarray(xperm.T), DCH).astype(bf16)  # [128, DCH, 2048]
        xo = np.ascontiguousarray(xperm).astype(bf16)
        # kv slots in FIXED pair order: [A blocks 0,3,4,7 | B blocks 1,2,5,6]
        kv_order = _ORDERS[0][:4] + _ORDERS[1][:4]

        def expM(gk_sub, gq_sub):
            diff = gk_sub[:, None] - gq_sub[None, :]
            m = np.where(diff <= 0, np.exp(_sigmoid(diff + pbm)), 0.0)
            return m.astype(np.float32)

        parts = []
        for qi in range(4):
            gq = np.arange(order[qi] * NBQ, (order[qi] + 1) * NBQ).astype(np.float64)
            for s in MASKED[qi]:
                gblk = kv_order[s]
                gk = np.arange(gblk * NBQ, (gblk + 1) * NBQ).astype(np.float64)
                # [512, 512] -> [128, 4, 512]
                parts.append(_chunk_part(expM(gk, gq), 4)[:, None, :, :])
        mskc = np.concatenate(parts, axis=1).astype(bf16)  # [128, NMSK, 4, 512]
        in_maps.append({
            "xt": xt, "xo": xo, "wkv": wkv, "wql": wql, "wproj": wproj,
            "msk": np.ascontiguousarray(mskc),
        })
    return in_maps


def kernel(x, expand, project, position_bias_mult):
    global LAST_RESULTS
    x = np.asarray(x, dtype=np.float32)
    expand = np.asarray(expand, dtype=np.float32)
    project = np.asarray(project, dtype=np.float32)
    pbm = float(np.asarray(position_bias_mult))

    in_maps = _prep_inputs(x, expand, project, pbm)
    nc = _build_nc()
    res = run_bass_kernel_spmd(nc, in_maps, core_ids=list(range(8)))
    LAST_RESULTS = res

    full = np.empty((B, N, D), dtype=np.float32)
    for c in range(8):
        b, half = c // 2, c % 2
        order = _ORDERS[half]
        o = res.results[c]["out"]
        for qi in range(4):
            blk = order[qi]
            full[b, blk * 512:(blk + 1) * 512] = o[qi * 512:(qi + 1) * 512]
    return full
